# revision 22
# baseline (speedup 1.0000x reference)
"""MoE linear kernel for Trainium2, 8 NeuronCores, data-parallel over batch.

Problem (hardcoded shapes):
  x  [8192, 1024] f32, Wg [1024, 16], bg [16], We [16, 1024, 1024], be [16, 1024]
  out[b, o] = sum_e softmax(x @ Wg + bg)[b, e] * (x @ We[e] + be[e])[b, o]

Strategy: data-parallel over batch (1024 rows/core), no collectives.
Host pre-packs inputs into the exact SBUF layouts (transposed x, per-expert
weight tiles) so every DMA is contiguous and the PE does only matmuls.

The active path is v3 (_emit_body_v3, used by kernel() when be == 0; the v2
seeded build remains as the be != 0 fallback). Everything bf16 (rel err
~3.5e-3, budget 2e-2). Per-core work is 2048 x [128x128] @ [128x512] bf16
matmuls = 437us at the 2.4 GHz PE roofline; the traced single-shot runs at
~94% PE-array occupancy with 216ns/MM (the hardware floor: 512 cols / 2.4GHz
+ ~3ns NX issue), so v3's wins over v2 (~473us -> ~465.5us) are at the
edges:
  - expert 0 runs kc-OUTER in three bt-groups with xe and wt0 DMA'd per-kc
    chunk on the scalar/sync queues, interleaved in emission order so the
    Tile scheduler's global-serial DMA model paces them correctly; the
    first expert matmuls issue at ~11us (preamble + first chunk pair)
    instead of ~22us (full xe upload + wt0 gap);
  - wg_s/bg_s emitted after the first chunk pair, else the scheduler
    hoists two gate chains to the top of the PE order where they stall
    ~1.5us each on late xe chunks;
  - the gate sits between expert 0's first and second bt-groups (pg in its
    own PSUM bank: sharing the expert ring would deadlock), expert PSUM
    ring of 7 banks, 4 We prefetch buffers (removes ~1.5us of fold-WAR and
    handoff gaps at the expert 0 -> 1 transition);
  - kc0's chunks split in half (the first expert MM needs only
    xe[:,0,0:128] + wt0[:,0,0:512]), so the PE un-gates at ~10.7us;
  - bf16 final folds so the out stores move half the bytes, one full-width
    store per bt alternating the two HW-DGE queues (scalar+sync) so the
    final stores drain in parallel (gpsimd's SW-DGE would add its 8 queue
    sems to the exit drain);
  - _thin_end_barrier drops the Tile end-block's redundant second
    all-engine barrier round (~1us).
Remaining measured overhead over the 442us floor, all framework/HW-level:
~7.5us engine preamble (bulk semaphore init + table loads), ~1.9us HAM
clock-gate ramp (9 cold MMs - the documented ~3.4us busy window; an
earlier dep-free warm-up burst did NOT move it and was removed), ~6.5us of
periodic single-MM dips to 379ns every 10.79us (= every 50 LDW+MM pairs =
6.4KB of instruction stream - consistent with NX instruction-fetch refill,
not addressable from the kernel), and ~11.9us exit tail: ~2us real
folds/stores/DMA-completion, the rest a walrus-level per-engine teardown
sequence (~60 serial 115ns semaphore micro-ops on the PE sequencer gate
the final barrier; they are not present in the emitted BIR).

Measurement note: per-MM rate flips between 216ns (2.4 GHz) and ~262ns
(~2.0 GHz chip downclock under sustained load) run to run; the prior
session's "sustained 266-277ns regardless of dtype/reuse/banks" was the
downclocked state, not an issue-rate limit. fp8 DoubleRow (2x PE rate) was
evaluated numerically and rejected: e4m3 quantization of both operands gives
3.9e-2 rel err vs the 2e-2 budget (even 4/16 experts in fp8 is 1.97e-2).

v2 notes (loop-bench era, kept for the seeded fallback):
  - gate computed directly from the bf16 x (no separate f32 x upload);
  - x double-buffered (xe_bufs=2) so the next loop iteration's x load
    overlaps the current expert loop;
  - when be is all-zero (the shipped problem), no gate transpose / be seed
    at all: the e=0 fold writes acc = psum*g directly, so the PE stream
    never waits on the softmax chain at iteration boundaries;
  - out stores on the Activation DMA queue; We streams on the sync queue;
  - unified 8-bank PSUM ring (per-(e,bt) pairs of 512-wide groups), DVE
    folds acc = psum * gate[:, e] + acc;
  - benchmark loop builds unroll 4 bodies per For_i iteration, for two
    reasons: (a) a For_i body is emitted once, so pool slots are fixed per
    iteration — with a single body the one-alloc-per-body xe tile reuses
    the same slot and its reload serializes the back-edge; unrolling makes
    odd-count tags genuinely alternate; (b) plain For_i runs an all-engine
    barrier + semaphore reset at every back edge (~20 us of drain), which
    unrolling amortizes. unroll=8 regresses badly (loop body exceeds the
    sequencers' instruction-fetch window); staggered_reset measured worse
    than the barrier it removes.

Measured: 594 us (v1 f32r baseline) -> ~552 us. The v1 f32r/bf16 paths and
the eg/fold2/ldw/psplit/staggered experiment knobs are kept for reference.
"""

import numpy as np

import concourse.bass as bass
import concourse.mybir as mybir
import concourse.tile as tile
from concourse.bass_utils import run_bass_kernel_spmd
from concourse.masks import make_identity

P = 128
B, D_IN, D_OUT, E = 8192, 1024, 1024, 16
NCORES = 8
BSH = B // NCORES          # 1024 batch rows per core
BT = BSH // P              # 8 batch tiles per core
KC = D_IN // P             # 8 contraction chunks
OH = 2                     # output halves
ON = D_OUT // OH           # 512 output cols per matmul group

F32 = mybir.dt.float32
F32R = mybir.dt.float32r
BF16 = mybir.dt.bfloat16

MM_DTYPE = "f32r"          # "f32r" | "bf16" for the expert matmuls


def _split_multi_waits(nc, limit=1):
    """The walrus build in this container rejects instructions carrying more
    than `limit` semaphore waits ("Too many sync wait commands" on the Tile
    tail drain). Move extra waits onto preceding same-engine NoOps."""
    n = 0
    for f in nc.m.functions:
        for bb in f.blocks:
            insts = bb.instructions
            i = 0
            while i < len(insts):
                ins = insts[i]
                si = ins.sync_info
                if si is not None and len(si.on_wait) > limit:
                    waits = list(si.on_wait)
                    extra, keep = waits[:-limit], waits[-limit:]
                    for j in range(0, len(extra), limit):
                        nop = mybir.InstNoOp(
                            name=f"I-waitsplit-{n}",
                            engine=ins.engine,
                            sync_info=mybir.SyncInfo(
                                on_wait=list(extra[j : j + limit]), on_update=[]
                            ),
                        )
                        n += 1
                        insts.insert(i, nop)
                        i += 1
                    si.on_wait = keep
                i += 1
    return n


def _emit_body(nc, pools, dram, mm_dtype, eg=1):
    persist, we_pool, sm_pool, psum = pools
    xtg, xte, wg, bgb, wep, be, out = dram
    EDT = F32R if mm_dtype == "f32r" else BF16

    ident = persist.tile([P, P], F32, tag="ident", name="ident")
    make_identity(nc, ident[:])

    # Small replicated tensors
    wg_s = persist.tile([P, KC, E], F32R, tag="wg", name="wg_s")
    nc.sync.dma_start(wg_s[:], wg.rearrange("(kc p) e -> p kc e", p=P).bitcast(F32R))
    bg_s = persist.tile([P, E], F32, tag="bg", name="bg_s")
    nc.sync.dma_start(bg_s[:], bgb[:])
    be_s = persist.tile([E, D_OUT], F32R, tag="be", name="be_s")
    nc.sync.dma_start(be_s[:], be[:].bitcast(F32R))

    # Transposed activations (pre-packed on host): [P, KC, BSH]
    # on the Activation-engine DMA queue so it overlaps the first We tile
    # load on the sync queue
    xg = persist.tile([P, KC, BSH], F32R, tag="xg", name="xg")
    nc.scalar.dma_start(xg[:], xtg[:].bitcast(F32R))
    if mm_dtype == "f32r":
        xe = xg
    else:
        xe = persist.tile([P, KC, BSH], EDT, tag="xe", name="xe")
        nc.sync.dma_start(xe[:], xte[:])

    gate = [
        persist.tile([P, E], F32, tag=f"g{bt}", name=f"g{bt}") for bt in range(BT)
    ]
    gateT = persist.tile([E, BSH], F32R, tag="gateT", name="gateT")
    acc = [
        [
            persist.tile([P, ON], F32, tag=f"acc{bt}_{oh}", name=f"acc{bt}_{oh}")
            for oh in range(OH)
        ]
        for bt in range(BT)
    ]

    # ---- Phase A: gate logits + softmax + gate^T ----
    for bt in range(BT):
        bsl = slice(bt * P, (bt + 1) * P)
        pg = psum.tile([P, E], F32, tag="ps", name="pg")
        for kc in range(KC):
            nc.tensor.matmul(
                pg[:],
                xg[:, kc, bsl],
                wg_s[:, kc, :],
                start=(kc == 0),
                stop=(kc == KC - 1),
            )
        logits = sm_pool.tile([P, E], F32, tag="logits", name="logits")
        nc.vector.tensor_add(logits[:], pg[:], bg_s[:])
        negmax = sm_pool.tile([P, 1], F32, tag="negmax", name="negmax")
        nc.vector.tensor_reduce(
            out=negmax[:],
            in_=logits[:],
            op=mybir.AluOpType.max,
            axis=mybir.AxisListType.X,
            negate=True,
        )
        esum = sm_pool.tile([P, 1], F32, tag="esum", name="esum")
        nc.scalar.activation(
            gate[bt][:],
            logits[:],
            mybir.ActivationFunctionType.Exp,
            bias=negmax[:, 0:1],
            accum_out=esum[:, 0:1],
        )
        rsum = sm_pool.tile([P, 1], F32, tag="rsum", name="rsum")
        nc.vector.reciprocal(rsum[:], esum[:])
        nc.vector.tensor_scalar_mul(gate[bt][:], gate[bt][:], rsum[:, 0:1])

        gtp = psum.tile([E, P], F32, tag="ps", name="gtp")
        nc.tensor.transpose(gtp[:], gate[bt][:], ident[:])
        nc.vector.tensor_copy(gateT[:, bsl], gtp[:])

    # ---- Phase A.5: seed accumulators with gate @ be ----
    for bt in range(BT):
        for oh in range(OH):
            psb = psum.tile([P, ON], F32, tag="ps", name="psb")
            nc.tensor.matmul(
                psb[:],
                gateT[:, bt * P : (bt + 1) * P],
                be_s[:, oh * ON : (oh + 1) * ON],
                start=True,
                stop=True,
            )
            nc.vector.tensor_copy(acc[bt][oh][:], psb[:])

    # ---- Phase B: expert loop (packed We streamed once) ----
    if eg == 1:
        for e in range(E):
            wt = we_pool.tile([P, KC, D_OUT], EDT, tag="we", name="wt")
            src = wep[e]
            nc.sync.dma_start(wt[:], src.bitcast(F32R) if mm_dtype == "f32r" else src)
            for bt in range(BT):
                bsl = slice(bt * P, (bt + 1) * P)
                ps = [
                    psum.tile([P, ON], F32, tag="ps", name=f"ps{oh}")
                    for oh in range(OH)
                ]
                for kc in range(KC):
                    for oh in range(OH):
                        # consecutive oh-pair shares the stationary operand
                        nc.tensor.matmul(
                            ps[oh][:],
                            xe[:, kc, bsl],
                            wt[:, kc, oh * ON : (oh + 1) * ON],
                            start=(kc == 0),
                            stop=(kc == KC - 1),
                        )
                for oh in range(OH):
                    nc.vector.scalar_tensor_tensor(
                        out=acc[bt][oh][:],
                        in0=ps[oh][:],
                        scalar=gate[bt][:, e : e + 1],
                        in1=acc[bt][oh][:],
                        op0=mybir.AluOpType.mult,
                        op1=mybir.AluOpType.add,
                    )
    else:
        # eg experts per group: the x-chunk stationary operand is shared
        # across eg*OH consecutive matmuls; eg*OH PSUM banks held per group.
        for eb in range(E // eg):
            wts = []
            for i in range(eg):
                wt = we_pool.tile([P, KC, D_OUT], EDT, tag=f"we{i}", name=f"wt{i}")
                src = wep[eb * eg + i]
                nc.sync.dma_start(
                    wt[:], src.bitcast(F32R) if mm_dtype == "f32r" else src
                )
                wts.append(wt)
            for bt in range(BT):
                bsl = slice(bt * P, (bt + 1) * P)
                ps = [
                    [
                        psum.tile([P, ON], F32, tag="ps", name=f"ps{i}_{oh}")
                        for oh in range(OH)
                    ]
                    for i in range(eg)
                ]
                for kc in range(KC):
                    for i in range(eg):
                        for oh in range(OH):
                            nc.tensor.matmul(
                                ps[i][oh][:],
                                xe[:, kc, bsl],
                                wts[i][:, kc, oh * ON : (oh + 1) * ON],
                                start=(kc == 0),
                                stop=(kc == KC - 1),
                            )
                for i in range(eg):
                    e = eb * eg + i
                    for oh in range(OH):
                        nc.vector.scalar_tensor_tensor(
                            out=acc[bt][oh][:],
                            in0=ps[i][oh][:],
                            scalar=gate[bt][:, e : e + 1],
                            in1=acc[bt][oh][:],
                            op0=mybir.AluOpType.mult,
                            op1=mybir.AluOpType.add,
                        )

    # ---- Phase C: store ----
    for bt in range(BT):
        for oh in range(OH):
            nc.sync.dma_start(
                out[bt * P : (bt + 1) * P, oh * ON : (oh + 1) * ON],
                acc[bt][oh][:],
            )


def _emit_body_v2(
    nc, pools, dram, seeded, fold2=False, ldw=False, psplit=False, chunk=False
):
    """bf16 everywhere: gate computed from the bf16 x directly (no f32 x
    upload), x double-buffered so its reload overlaps the previous
    iteration's expert loop, out stores on the vector DMA queue so they
    don't delay the next iteration's We streaming on the sync queue.

    seeded=False (be known all-zero): no gate transpose / be seed at all —
    the e=0 fold writes acc = ps*g directly, so the PE never waits on the
    softmax chain and iterations butt up back-to-back."""
    persist, xe_pool, we_pool, sm_pool, psum = pools
    xte, wg, bgb, wep, be, out = dram

    wg_s = persist.tile([P, KC, E], BF16, tag="wg", name="wg_s")
    nc.sync.dma_start(wg_s[:], wg.rearrange("(kc p) e -> p kc e", p=P))
    bg_s = persist.tile([P, E], F32, tag="bg", name="bg_s")
    nc.sync.dma_start(bg_s[:], bgb[:])
    if seeded:
        ident = persist.tile([P, P], F32, tag="ident", name="ident")
        make_identity(nc, ident[:])
        be_s = persist.tile([E, D_OUT], F32R, tag="be", name="be_s")
        nc.sync.dma_start(be_s[:], be[:].bitcast(F32R))

    # x transposed [P, KC, BSH] bf16, double-buffered across iterations;
    # on the Activation-engine DMA queue to overlap We streaming.
    # Loaded per-kc chunk so the first gate matmul (which only needs
    # chunk 0) starts ~5us earlier on a cold start — subtile dep
    # tracking scopes each matmul's wait to its chunk.
    xe = xe_pool.tile([P, KC, BSH], BF16, tag="xe", name="xe")
    if chunk:
        for kc in range(KC):
            nc.scalar.dma_start(xe[:, kc, :], xte[:, kc, :])
    else:
        nc.scalar.dma_start(xe[:], xte[:])

    gate = [
        persist.tile([P, E], F32, tag=f"g{bt}", name=f"g{bt}") for bt in range(BT)
    ]
    if fold2:
        acc = [
            persist.tile([P, D_OUT], F32, tag=f"acc{bt}", name=f"acc{bt}")
            for bt in range(BT)
        ]
    else:
        acc = [
            [
                persist.tile([P, ON], F32, tag=f"acc{bt}_{oh}", name=f"acc{bt}_{oh}")
                for oh in range(OH)
            ]
            for bt in range(BT)
        ]
    if seeded:
        gateT = persist.tile([E, BSH], F32R, tag="gateT", name="gateT")

    if fold2:
        pg_tag, pg_bufs = "pg", 2
    elif psplit:
        # pg gets its own bank so the next iteration's gate matmuls never
        # WAR against the previous body's tail folds in the phase-B ring
        pg_tag, pg_bufs = "pg", 1
    else:
        pg_tag, pg_bufs = "ps", None
    ps_bufs = 7 if psplit else None

    # ---- Phase A: gate logits + softmax ----
    for bt in range(BT):
        bsl = slice(bt * P, (bt + 1) * P)
        pg = psum.tile([P, E], F32, tag=pg_tag, name="pg", bufs=pg_bufs)
        for kc in range(KC):
            nc.tensor.matmul(
                pg[:],
                xe[:, kc, bsl],
                wg_s[:, kc, :],
                start=(kc == 0),
                stop=(kc == KC - 1),
            )
        logits = sm_pool.tile([P, E], F32, tag="logits", name="logits")
        nc.vector.tensor_add(logits[:], pg[:], bg_s[:])
        negmax = sm_pool.tile([P, 1], F32, tag="negmax", name="negmax")
        nc.vector.tensor_reduce(
            out=negmax[:],
            in_=logits[:],
            op=mybir.AluOpType.max,
            axis=mybir.AxisListType.X,
            negate=True,
        )
        esum = sm_pool.tile([P, 1], F32, tag="esum", name="esum")
        nc.scalar.activation(
            gate[bt][:],
            logits[:],
            mybir.ActivationFunctionType.Exp,
            bias=negmax[:, 0:1],
            accum_out=esum[:, 0:1],
        )
        rsum = sm_pool.tile([P, 1], F32, tag="rsum", name="rsum")
        nc.vector.reciprocal(rsum[:], esum[:])
        nc.vector.tensor_scalar_mul(gate[bt][:], gate[bt][:], rsum[:, 0:1])

        if seeded:
            gtp = psum.tile([E, P], F32, tag="ps", name="gtp")
            nc.tensor.transpose(gtp[:], gate[bt][:], ident[:])
            nc.vector.tensor_copy(gateT[:, bsl], gtp[:])

    # ---- Phase A.5: seed accumulators with gate @ be ----
    if seeded:
        for bt in range(BT):
            for oh in range(OH):
                psb = psum.tile([P, ON], F32, tag="ps", name="psb")
                nc.tensor.matmul(
                    psb[:],
                    gateT[:, bt * P : (bt + 1) * P],
                    be_s[:, oh * ON : (oh + 1) * ON],
                    start=True,
                    stop=True,
                )
                nc.vector.tensor_copy(acc[bt][oh][:], psb[:])

    # ---- Phase B: expert loop ----
    for e in range(E):
        wt = we_pool.tile([P, KC, D_OUT], BF16, tag="we", name="wt")
        if chunk:
            # two-half load: kc 0-3 arrive first so the expert's first
            # matmuls can start while the second half streams
            nc.sync.dma_start(wt[:, 0 : KC // 2, :], wep[e, :, 0 : KC // 2, :])
            nc.sync.dma_start(wt[:, KC // 2 :, :], wep[e, :, KC // 2 :, :])
        else:
            nc.sync.dma_start(wt[:], wep[e])
        for bt in range(BT):
            bsl = slice(bt * P, (bt + 1) * P)
            if fold2:
                # one 2-bank PSUM tile per (e, bt); each matmul targets a
                # bank-aligned 512-wide half, the fold covers both at once
                ps2 = psum.tile([P, D_OUT], F32, tag="ps2", name="ps2", bufs=3)
                ps = [ps2[:, oh * ON : (oh + 1) * ON] for oh in range(OH)]
            else:
                ps = [
                    psum.tile([P, ON], F32, tag="ps", name=f"ps{oh}", bufs=ps_bufs)[:]
                    for oh in range(OH)
                ]
            for kc in range(KC):
                if ldw:
                    # one explicit stationary load per (bt, kc); the oh-pair
                    # matmuls skip their self-load (uses the loaded weights)
                    nc.tensor.ldweights(xe[:, kc, bsl])
                for oh in range(OH):
                    mm = nc.tensor.matmul(
                        ps[oh],
                        xe[:, kc, bsl],
                        wt[:, kc, oh * ON : (oh + 1) * ON],
                        start=(kc == 0),
                        stop=(kc == KC - 1),
                    )
                    if ldw:
                        mm.ins.ldweights = False
            if fold2:
                if e == 0 and not seeded:
                    nc.vector.tensor_scalar_mul(
                        acc[bt][:], ps2[:], gate[bt][:, 0:1]
                    )
                else:
                    nc.vector.scalar_tensor_tensor(
                        out=acc[bt][:],
                        in0=ps2[:],
                        scalar=gate[bt][:, e : e + 1],
                        in1=acc[bt][:],
                        op0=mybir.AluOpType.mult,
                        op1=mybir.AluOpType.add,
                    )
            else:
                for oh in range(OH):
                    if e == 0 and not seeded:
                        nc.vector.tensor_scalar_mul(
                            acc[bt][oh][:], ps[oh], gate[bt][:, 0:1]
                        )
                    else:
                        nc.vector.scalar_tensor_tensor(
                            out=acc[bt][oh][:],
                            in0=ps[oh],
                            scalar=gate[bt][:, e : e + 1],
                            in1=acc[bt][oh][:],
                            op0=mybir.AluOpType.mult,
                            op1=mybir.AluOpType.add,
                        )

    # ---- Phase C: store (Activation DMA queue; sync queue keeps We) ----
    for bt in range(BT):
        if fold2:
            nc.scalar.dma_start(out[bt * P : (bt + 1) * P, :], acc[bt][:])
        else:
            for oh in range(OH):
                nc.scalar.dma_start(
                    out[bt * P : (bt + 1) * P, oh * ON : (oh + 1) * ON],
                    acc[bt][oh][:],
                )


def _thin_end_barrier(nc):
    """Drop the TileContext end-block's second all-engine barrier round.

    The exit sequence is: DMA-queue completion waits, a gather/release
    barrier (engines quiesced + synchronized), Pool's ucode teardown
    (InstISA), then a SECOND identical barrier round before the engines
    fall off the end of their streams. The entry preamble bulk-resets all
    semaphores, so the second round buys nothing for a kernel-dev NEFF;
    removing it shaves ~1-2us off the measured execution span. Verified by
    re-executing the same compiled NEFF repeatedly (outputs stable).
    """
    for f in nc.m.functions:
        for bb in f.blocks:
            if not bb.name.endswith("_end"):
                continue
            insts = bb.instructions
            isa_idx = [
                k for k, ins in enumerate(insts)
                if type(ins).__name__ == "InstISA"
            ]
            if not isa_idx:
                continue
            cut = isa_idx[-1] + 1
            trailing = insts[cut:]
            if trailing and all(
                type(t).__name__ in ("InstDrain", "InstEventSemaphore", "InstNoOp")
                for t in trailing
            ):
                del insts[cut:]
    return nc


def _emit_body_v3(nc, pools, dram):
    """Single-shot-optimized unseeded body (be == 0).

    v2's single-shot trace: first MM at 16.4us (preamble + full 2MB xe DMA),
    a 3.7us wt0-wait gap after the gate, ~6us HAM cold-ramp penalty, then a
    gapless 220ns/MM stream (hardware floor), 12.3us tail. v3 attacks the
    edges; the MM stream itself is already at the bf16 roofline:
      - expert 0 runs kc-OUTER in three bt-groups (3/3/2), with xe and wt0
        both DMA'd per-kc chunk on separate queues: the first MMs issue as
        soon as chunk 0 lands (~6us, preamble-bound) instead of after the
        full xe upload, and the HAM warm-up ramp is absorbed by real work;
      - the gate (needs all xe chunks) moves between expert-0's first and
        second bt-groups: by then xe is resident, so the PE never waits on
        it. pg gets its own 2-bank PSUM tag: sharing the expert ring would
        create a WAR cycle (gate MM waits fold that waits gate) = deadlock;
      - experts 1..15 keep the v2 shape (bt outer, kc inner, oh pair) so
        per-bt folds stay staggered and the final expert's exposed tail is
        only bt=7's two folds;
      - out stores alternate the scalar/vector DMA queues so the last two
        stores drain in parallel.
    """
    persist, we_pool, sm_pool, psum = pools
    xte, wg, bgb, wep, out = dram

    # xe per-kc chunks on the scalar queue; wt0 per-kc chunks on the sync
    # queue: the (xe, wt0) chunk pair for each kc streams on two queues in
    # parallel, pacing expert 0's kc-outer MM groups. The emission MUST
    # interleave (xe_k, wt0_k): Tile's scheduler simulates all DMA queues as
    # one exclusive global device in instruction-emission order, so emitting
    # all xe chunks first makes it believe wt0 lands after the entire xe and
    # it then statically orders the gate matmuls (which need all of xe)
    # ahead of expert 0 — on real HW that ordering blocks the PE queue until
    # the full 2MB xe upload (~13us) instead of the first chunk pair (~6us).
    # wg_s/bg_s are emitted AFTER the first chunk pair for the same reason:
    # with wg_s first on the sync queue, the sim thinks the gate's operands
    # land before expert 0's and hoists two gate chains to the top of the PE
    # order, where on HW they stall ~1.5us each on late xe chunks.
    xe = persist.tile([P, KC, BSH], BF16, tag="xe", name="xe")
    wt0 = we_pool.tile([P, KC, D_OUT], BF16, tag="we", name="wt0")
    wg_s = persist.tile([P, KC, E], BF16, tag="wg", name="wg_s")
    bg_s = persist.tile([P, E], F32, tag="bg", name="bg_s")
    H = BSH // 2
    for kc in range(KC):
        if kc == 0:
            # kc0 in half chunks: the first expert MM needs only
            # xe[:, 0, 0:128] and wt0[:, 0, 0:512], so a 128KB first
            # transfer un-gates the PE ~2us sooner than a 256KB one
            nc.scalar.dma_start(xe[:, 0, 0:H], xte[:, 0, 0:H])
            nc.sync.dma_start(wt0[:, 0, 0:H], wep[0, :, 0, 0:H])
            nc.scalar.dma_start(xe[:, 0, H:], xte[:, 0, H:])
            nc.sync.dma_start(wt0[:, 0, H:], wep[0, :, 0, H:])
            nc.sync.dma_start(wg_s[:], wg.rearrange("(kc p) e -> p kc e", p=P))
            nc.sync.dma_start(bg_s[:], bgb[:])
        else:
            nc.scalar.dma_start(xe[:, kc, :], xte[:, kc, :])
            nc.sync.dma_start(wt0[:, kc, :], wep[0, :, kc, :])

    gate = [
        persist.tile([P, E], F32, tag=f"g{bt}", name=f"g{bt}") for bt in range(BT)
    ]
    acc = [
        [
            persist.tile([P, ON], F32, tag=f"acc{bt}_{oh}", name=f"acc{bt}_{oh}")
            for oh in range(OH)
        ]
        for bt in range(BT)
    ]
    # final fold (e = E-1) writes bf16 so the out stores move half the bytes;
    # intermediate accumulation stays f32. One full-width tile per bt so the
    # store is a single descriptor (8 stores instead of 16: fewer serial
    # ~0.6us descriptor slots and completion waits in the exit drain).
    accf = [
        persist.tile([P, D_OUT], BF16, tag=f"af{bt}", name=f"af{bt}")
        for bt in range(BT)
    ]

    def expert0_group(bts):
        ps = {
            (bt, oh): psum.tile([P, ON], F32, tag="ps", name=f"ps{bt}_{oh}", bufs=7)
            for bt in bts
            for oh in range(OH)
        }
        for kc in range(KC):
            for bt in bts:
                bsl = slice(bt * P, (bt + 1) * P)
                for oh in range(OH):
                    nc.tensor.matmul(
                        ps[bt, oh][:],
                        xe[:, kc, bsl],
                        wt0[:, kc, oh * ON : (oh + 1) * ON],
                        start=(kc == 0),
                        stop=(kc == KC - 1),
                    )
        return ps

    def expert0_folds(bts, ps):
        for bt in bts:
            for oh in range(OH):
                nc.vector.tensor_scalar_mul(
                    acc[bt][oh][:], ps[bt, oh][:], gate[bt][:, 0:1]
                )

    def emit_gate():
        for bt in range(BT):
            bsl = slice(bt * P, (bt + 1) * P)
            pg = psum.tile([P, E], F32, tag="pg", name="pg", bufs=1)
            for kc in range(KC):
                nc.tensor.matmul(
                    pg[:],
                    xe[:, kc, bsl],
                    wg_s[:, kc, :],
                    start=(kc == 0),
                    stop=(kc == KC - 1),
                )
            logits = sm_pool.tile([P, E], F32, tag="logits", name="logits")
            nc.vector.tensor_add(logits[:], pg[:], bg_s[:])
            negmax = sm_pool.tile([P, 1], F32, tag="negmax", name="negmax")
            nc.vector.tensor_reduce(
                out=negmax[:],
                in_=logits[:],
                op=mybir.AluOpType.max,
                axis=mybir.AxisListType.X,
                negate=True,
            )
            esum = sm_pool.tile([P, 1], F32, tag="esum", name="esum")
            nc.scalar.activation(
                gate[bt][:],
                logits[:],
                mybir.ActivationFunctionType.Exp,
                bias=negmax[:, 0:1],
                accum_out=esum[:, 0:1],
            )
            rsum = sm_pool.tile([P, 1], F32, tag="rsum", name="rsum")
            nc.vector.reciprocal(rsum[:], esum[:])
            nc.vector.tensor_scalar_mul(gate[bt][:], gate[bt][:], rsum[:, 0:1])

    # ---- expert 0, DMA-chunk-paced, with the gate between groups A and B ----
    psA = expert0_group((0, 1, 2))
    emit_gate()
    expert0_folds((0, 1, 2), psA)
    psB = expert0_group((3, 4, 5))
    expert0_folds((3, 4, 5), psB)
    psC = expert0_group((6, 7))
    expert0_folds((6, 7), psC)

    # ---- experts 1..15, v2 shape ----
    for e in range(1, E):
        wt = we_pool.tile([P, KC, D_OUT], BF16, tag="we", name="wt")
        nc.sync.dma_start(wt[:], wep[e])
        for bt in range(BT):
            bsl = slice(bt * P, (bt + 1) * P)
            ps = [
                psum.tile([P, ON], F32, tag="ps", name=f"ps{oh}", bufs=7)
                for oh in range(OH)
            ]
            for kc in range(KC):
                for oh in range(OH):
                    nc.tensor.matmul(
                        ps[oh][:],
                        xe[:, kc, bsl],
                        wt[:, kc, oh * ON : (oh + 1) * ON],
                        start=(kc == 0),
                        stop=(kc == KC - 1),
                    )
            for oh in range(OH):
                dst = (
                    accf[bt][:, oh * ON : (oh + 1) * ON]
                    if e == E - 1
                    else acc[bt][oh][:]
                )
                nc.vector.scalar_tensor_tensor(
                    out=dst,
                    in0=ps[oh][:],
                    scalar=gate[bt][:, e : e + 1],
                    in1=acc[bt][oh][:],
                    op0=mybir.AluOpType.mult,
                    op1=mybir.AluOpType.add,
                )

    # ---- stores: one full-width store per bt, alternating the two HW-DGE
    # queues (scalar + sync; sync is idle once the weights are in) so the
    # final two stores drain in parallel. gpsimd's SW-DGE is avoided: it
    # would add its 8 DMASW queue-completion semaphores to the exit drain.
    for bt in range(BT):
        q = nc.scalar if bt % 2 == 0 else nc.sync
        q.dma_start(out[bt * P : (bt + 1) * P, :], accf[bt][:])


def _build_v3():
    nc = bass.Bass(trn_type="TRN2")
    xte = nc.dram_tensor("xte", [P, KC, BSH], BF16, kind="ExternalInput")
    wg = nc.dram_tensor("wg", [D_IN, E], BF16, kind="ExternalInput")
    bgb = nc.dram_tensor("bgb", [P, E], F32, kind="ExternalInput")
    wep = nc.dram_tensor("wep", [E, P, KC, D_OUT], BF16, kind="ExternalInput")
    out = nc.dram_tensor("out", [BSH, D_OUT], BF16, kind="ExternalOutput")
    dram = (xte, wg, bgb, wep, out)

    with tile.TileContext(nc) as tc:
        with (
            tc.tile_pool(name="persist", bufs=1) as persist,
            tc.tile_pool(name="wes", bufs=4) as we_pool,
            tc.tile_pool(name="sm", bufs=2) as sm_pool,
            tc.tile_pool(name="psum", bufs=7, space="PSUM") as psum,
        ):
            _emit_body_v3(nc, (persist, we_pool, sm_pool, psum), dram)

    _split_multi_waits(nc)
    _thin_end_barrier(nc)
    return nc


def _build_v2(
    repeat=1,
    loop_n=1,
    we_bufs=3,
    xe_bufs=2,
    seeded=True,
    fold2=False,
    ldw=False,
    unroll=1,
    psplit=False,
    staggered=False,
    # chunked initial loads help a cold single-shot start (~5-10us in sim)
    # but cost ~4us/iter in the loop (more DMA insts + subtile sems) —
    # measured in-run A/B; the loop slope is the graded number, so off.
    chunk=False,
    hints=False,
):
    assert not (fold2 and seeded), "fold2 path only implemented for be==0"
    nc = bass.Bass(trn_type="TRN2")
    xte = nc.dram_tensor("xte", [P, KC, BSH], BF16, kind="ExternalInput")
    wg = nc.dram_tensor("wg", [D_IN, E], BF16, kind="ExternalInput")
    bgb = nc.dram_tensor("bgb", [P, E], F32, kind="ExternalInput")
    wep = nc.dram_tensor("wep", [E, P, KC, D_OUT], BF16, kind="ExternalInput")
    be = (
        nc.dram_tensor("be", [E, D_OUT], F32, kind="ExternalInput")
        if seeded
        else None
    )
    out = nc.dram_tensor("out", [BSH, D_OUT], F32, kind="ExternalOutput")
    dram = (xte, wg, bgb, wep, be, out)

    with tile.TileContext(nc) as tc:
        with (
            tc.tile_pool(name="persist", bufs=1) as persist,
            tc.tile_pool(name="xep", bufs=xe_bufs) as xe_pool,
            tc.tile_pool(name="wes", bufs=we_bufs) as we_pool,
            tc.tile_pool(name="sm", bufs=2) as sm_pool,
            tc.tile_pool(name="psum", bufs=8, space="PSUM") as psum,
        ):
            pools = (persist, xe_pool, we_pool, sm_pool, psum)
            if loop_n > 1:
                # unroll>1 makes odd-buffered pools (xe) genuinely alternate
                # across bodies: a For_i body is emitted once, so buffer
                # slots are fixed per iteration — with one body per
                # iteration the xe reload WARs against the same iteration's
                # last matmul and serializes the loop back-edge.
                # staggered_reset removes the per-iteration all-engine
                # barrier in the loop's semaphore-reset block (back-edge
                # jumps straight to the body; sems reset in stage preambles)
                # hints: back-edge branch-prefetch so each sequencer
                # prefetches the loop-body target before branching back.
                # True/"start": hint at body start; "late": before the last
                # unrolled body (closer to the branch, so the prefetched
                # target isn't re-evicted); "both": both sites.
                hint_kw = (
                    dict(
                        hint_engines=tuple(mybir.ALL_ENGINES),
                        back_edge_label="body_start",
                    )
                    if hints
                    else {}
                )
                with tc.For_i(0, loop_n, 1, staggered_reset=staggered, **hint_kw):
                    for u in range(unroll):
                        at_start = u == 0 and hints in (True, "start", "both")
                        at_late = u == unroll - 1 and hints in ("late", "both")
                        if at_start or at_late:
                            tc.mark_branch_hint_location(
                                "body_start", engines=tuple(mybir.ALL_ENGINES)
                            )
                        _emit_body_v2(
                            nc, pools, dram, seeded, fold2=fold2, ldw=ldw,
                            psplit=psplit, chunk=chunk,
                        )
            else:
                for _ in range(repeat):
                    _emit_body_v2(
                        nc, pools, dram, seeded, fold2=fold2, ldw=ldw,
                        psplit=psplit, chunk=chunk,
                    )

    _split_multi_waits(nc)
    return nc


def make_in_maps_v2(x, Wg, bg, We, be, seeded=True):
    import ml_dtypes

    x = np.ascontiguousarray(np.asarray(x, dtype=np.float32))
    Wg = np.ascontiguousarray(np.asarray(Wg, dtype=np.float32))
    bg = np.asarray(bg, dtype=np.float32).reshape(E)
    We = np.ascontiguousarray(np.asarray(We, dtype=np.float32))
    be = np.ascontiguousarray(np.asarray(be, dtype=np.float32))
    bgb = np.ascontiguousarray(np.broadcast_to(bg[None, :], (P, E)))

    wep = np.ascontiguousarray(
        We.reshape(E, KC, P, D_OUT).transpose(0, 2, 1, 3)
    ).astype(ml_dtypes.bfloat16)
    wgb = Wg.astype(ml_dtypes.bfloat16)

    in_maps = []
    for c in range(NCORES):
        xs = x[c * BSH : (c + 1) * BSH]
        xt = np.ascontiguousarray(xs.reshape(BSH, KC, P).transpose(2, 1, 0)).astype(
            ml_dtypes.bfloat16
        )
        m = {"xte": xt, "wg": wgb, "bgb": bgb, "wep": wep}
        if seeded:
            m["be"] = be
        in_maps.append(m)
    return in_maps


def _build(repeat=1, loop_n=1, mm_dtype=MM_DTYPE, psum_bufs=8, we_bufs=3, eg=1):
    nc = bass.Bass(trn_type="TRN2")
    EDT = F32R if mm_dtype == "f32r" else BF16

    # Host-packed inputs (see make_in_maps):
    #   xtg: x shard transposed  [P, KC, BSH] f32 (gate path, fp32r view)
    #   xte: same in bf16 (expert path; only uploaded for bf16 variant)
    #   wep: We packed [E, P, KC, D_OUT] in expert dtype
    xtg = nc.dram_tensor("xtg", [P, KC, BSH], F32, kind="ExternalInput")
    xte = (
        nc.dram_tensor("xte", [P, KC, BSH], BF16, kind="ExternalInput")
        if mm_dtype == "bf16"
        else None
    )
    wg = nc.dram_tensor("wg", [D_IN, E], F32, kind="ExternalInput")
    bgb = nc.dram_tensor("bgb", [P, E], F32, kind="ExternalInput")
    wep = nc.dram_tensor(
        "wep", [E, P, KC, D_OUT], F32 if mm_dtype == "f32r" else BF16,
        kind="ExternalInput",
    )
    be = nc.dram_tensor("be", [E, D_OUT], F32, kind="ExternalInput")
    out = nc.dram_tensor("out", [BSH, D_OUT], F32, kind="ExternalOutput")
    dram = (xtg, xte, wg, bgb, wep, be, out)

    with tile.TileContext(nc) as tc:
        with (
            tc.tile_pool(name="persist", bufs=1) as persist,
            tc.tile_pool(name="wes", bufs=we_bufs) as we_pool,
            tc.tile_pool(name="sm", bufs=2) as sm_pool,
            tc.tile_pool(name="psum", bufs=psum_bufs, space="PSUM") as psum,
        ):
            pools = (persist, we_pool, sm_pool, psum)
            if loop_n > 1:
                with tc.For_i(0, loop_n, 1):
                    _emit_body(nc, pools, dram, mm_dtype, eg=eg)
            else:
                for _ in range(repeat):
                    _emit_body(nc, pools, dram, mm_dtype, eg=eg)

    _split_multi_waits(nc)
    return nc


_CACHE = {}


def _get_nc(repeat=1, **kw):
    key = ("nc", repeat, tuple(sorted(kw.items())))
    if key not in _CACHE:
        _CACHE[key] = _build(repeat, **kw)
    return _CACHE[key]


def make_in_maps(x, Wg, bg, We, be, mm_dtype=MM_DTYPE):
    import ml_dtypes

    x = np.ascontiguousarray(np.asarray(x, dtype=np.float32))
    Wg = np.ascontiguousarray(np.asarray(Wg, dtype=np.float32))
    bg = np.asarray(bg, dtype=np.float32).reshape(E)
    We = np.ascontiguousarray(np.asarray(We, dtype=np.float32))
    be = np.ascontiguousarray(np.asarray(be, dtype=np.float32))
    bgb = np.ascontiguousarray(np.broadcast_to(bg[None, :], (P, E)))

    # We packed to [E, P, KC, D_OUT]: wep[e, p, kc, o] = We[e, kc*P+p, o]
    wep = np.ascontiguousarray(
        We.reshape(E, KC, P, D_OUT).transpose(0, 2, 1, 3)
    )
    if mm_dtype == "bf16":
        wep = wep.astype(ml_dtypes.bfloat16)

    in_maps = []
    for c in range(NCORES):
        xs = x[c * BSH : (c + 1) * BSH]
        # xT packed to [P, KC, BSH]: xt[p, kc, b] = xs[b, kc*P+p]
        xt = np.ascontiguousarray(xs.reshape(BSH, KC, P).transpose(2, 1, 0))
        m = {"xtg": xt, "wg": Wg, "bgb": bgb, "wep": wep, "be": be}
        if mm_dtype == "bf16":
            m["xte"] = xt.astype(ml_dtypes.bfloat16)
        in_maps.append(m)
    return in_maps


def _get_nc_v2(repeat=1, **kw):
    key = ("v2", repeat, tuple(sorted(kw.items())))
    if key not in _CACHE:
        _CACHE[key] = _build_v2(repeat, **kw)
    return _CACHE[key]


def _get_nc_v3():
    key = ("v3",)
    if key not in _CACHE:
        _CACHE[key] = _build_v3()
    return _CACHE[key]


def kernel(x, Wg, bg, We, be):
    seeded = bool(np.asarray(be, dtype=np.float32).any())
    in_maps = make_in_maps_v2(x, Wg, bg, We, be, seeded=seeded)
    nc = _get_nc_v2(seeded=True) if seeded else _get_nc_v3()
    res = run_bass_kernel_spmd(nc, in_maps, core_ids=list(range(NCORES)))
    return np.concatenate(
        [np.asarray(r["out"]).astype(np.float32) for r in res.results], axis=0
    )



# revision 24
# speedup vs baseline: 1.0014x; 1.0014x over previous
"""MoE linear kernel for Trainium2, 8 NeuronCores, data-parallel over batch.

Problem (hardcoded shapes):
  x  [8192, 1024] f32, Wg [1024, 16], bg [16], We [16, 1024, 1024], be [16, 1024]
  out[b, o] = sum_e softmax(x @ Wg + bg)[b, e] * (x @ We[e] + be[e])[b, o]

Strategy: data-parallel over batch (1024 rows/core), no collectives.
Host pre-packs inputs into the exact SBUF layouts (transposed x, per-expert
weight tiles) so every DMA is contiguous and the PE does only matmuls.

The active path is v3 (_emit_body_v3, used by kernel() when be == 0; the v2
seeded build remains as the be != 0 fallback). Everything bf16 (rel err
~3.5e-3, budget 2e-2). Per-core work is 2048 x [128x128] @ [128x512] bf16
matmuls = 437us at the 2.4 GHz PE roofline; the traced single-shot runs at
~94% PE-array occupancy with 216ns/MM (the hardware floor: 512 cols / 2.4GHz
+ ~3ns NX issue), so v3's wins over v2 (~473us -> ~465.5us) are at the
edges:
  - expert 0 runs kc-OUTER in three bt-groups with xe and wt0 DMA'd per-kc
    chunk on the scalar/sync queues, interleaved in emission order so the
    Tile scheduler's global-serial DMA model paces them correctly; the
    first expert matmuls issue at ~11us (preamble + first chunk pair)
    instead of ~22us (full xe upload + wt0 gap);
  - wg_s/bg_s emitted after the first chunk pair, else the scheduler
    hoists two gate chains to the top of the PE order where they stall
    ~1.5us each on late xe chunks;
  - the gate sits between expert 0's first and second bt-groups (pg in its
    own PSUM bank: sharing the expert ring would deadlock), expert PSUM
    ring of 7 banks, 4 We prefetch buffers (removes ~1.5us of fold-WAR and
    handoff gaps at the expert 0 -> 1 transition);
  - kc0's chunks split in half (the first expert MM needs only
    xe[:,0,0:128] + wt0[:,0,0:512]), so the PE un-gates at ~10.7us;
  - bf16 final folds so the out stores move half the bytes, one full-width
    store per bt alternating the two HW-DGE queues (scalar+sync) so the
    final stores drain in parallel (gpsimd's SW-DGE would add its 8 queue
    sems to the exit drain); the very last (e,bt) pair runs its oh halves
    sequentially with its store split across both queues, so the exposed
    tail after the final matmul is one half-width fold + one half-width
    store (~0.8us saved);
  - _thin_end_barrier drops the Tile end-block's redundant second
    all-engine barrier round (~1us).
Remaining measured overhead over the 442us floor, all framework/HW-level:
~7.5us engine preamble (bulk semaphore init + table loads), ~1.9us HAM
clock-gate ramp (9 cold MMs - the documented ~3.4us busy window; an
earlier dep-free warm-up burst did NOT move it and was removed), ~6.5us of
periodic single-MM dips to 379ns every 10.79us (= every 50 LDW+MM pairs =
6.4KB of instruction stream - consistent with NX instruction-fetch refill,
not addressable from the kernel), and ~11.9us exit tail: ~2us real
folds/stores/DMA-completion, the rest a walrus-level per-engine teardown
sequence (~60 serial 115ns semaphore micro-ops on the PE sequencer gate
the final barrier; they are not present in the emitted BIR).

Measurement note: per-MM rate flips between 216ns (2.4 GHz) and ~262ns
(~2.0 GHz chip downclock under sustained load) run to run; the prior
session's "sustained 266-277ns regardless of dtype/reuse/banks" was the
downclocked state, not an issue-rate limit. fp8 DoubleRow (2x PE rate) was
evaluated numerically and rejected: e4m3 quantization of both operands gives
3.9e-2 rel err vs the 2e-2 budget (even 4/16 experts in fp8 is 1.97e-2).

v2 notes (loop-bench era, kept for the seeded fallback):
  - gate computed directly from the bf16 x (no separate f32 x upload);
  - x double-buffered (xe_bufs=2) so the next loop iteration's x load
    overlaps the current expert loop;
  - when be is all-zero (the shipped problem), no gate transpose / be seed
    at all: the e=0 fold writes acc = psum*g directly, so the PE stream
    never waits on the softmax chain at iteration boundaries;
  - out stores on the Activation DMA queue; We streams on the sync queue;
  - unified 8-bank PSUM ring (per-(e,bt) pairs of 512-wide groups), DVE
    folds acc = psum * gate[:, e] + acc;
  - benchmark loop builds unroll 4 bodies per For_i iteration, for two
    reasons: (a) a For_i body is emitted once, so pool slots are fixed per
    iteration — with a single body the one-alloc-per-body xe tile reuses
    the same slot and its reload serializes the back-edge; unrolling makes
    odd-count tags genuinely alternate; (b) plain For_i runs an all-engine
    barrier + semaphore reset at every back edge (~20 us of drain), which
    unrolling amortizes. unroll=8 regresses badly (loop body exceeds the
    sequencers' instruction-fetch window); staggered_reset measured worse
    than the barrier it removes.

Measured: 594 us (v1 f32r baseline) -> ~552 us. The v1 f32r/bf16 paths and
the eg/fold2/ldw/psplit/staggered experiment knobs are kept for reference.
"""

import numpy as np

import concourse.bass as bass
import concourse.mybir as mybir
import concourse.tile as tile
from concourse.bass_utils import run_bass_kernel_spmd
from concourse.masks import make_identity

P = 128
B, D_IN, D_OUT, E = 8192, 1024, 1024, 16
NCORES = 8
BSH = B // NCORES          # 1024 batch rows per core
BT = BSH // P              # 8 batch tiles per core
KC = D_IN // P             # 8 contraction chunks
OH = 2                     # output halves
ON = D_OUT // OH           # 512 output cols per matmul group

F32 = mybir.dt.float32
F32R = mybir.dt.float32r
BF16 = mybir.dt.bfloat16

MM_DTYPE = "f32r"          # "f32r" | "bf16" for the expert matmuls


def _split_multi_waits(nc, limit=1):
    """The walrus build in this container rejects instructions carrying more
    than `limit` semaphore waits ("Too many sync wait commands" on the Tile
    tail drain). Move extra waits onto preceding same-engine NoOps."""
    n = 0
    for f in nc.m.functions:
        for bb in f.blocks:
            insts = bb.instructions
            i = 0
            while i < len(insts):
                ins = insts[i]
                si = ins.sync_info
                if si is not None and len(si.on_wait) > limit:
                    waits = list(si.on_wait)
                    extra, keep = waits[:-limit], waits[-limit:]
                    for j in range(0, len(extra), limit):
                        nop = mybir.InstNoOp(
                            name=f"I-waitsplit-{n}",
                            engine=ins.engine,
                            sync_info=mybir.SyncInfo(
                                on_wait=list(extra[j : j + limit]), on_update=[]
                            ),
                        )
                        n += 1
                        insts.insert(i, nop)
                        i += 1
                    si.on_wait = keep
                i += 1
    return n


def _emit_body(nc, pools, dram, mm_dtype, eg=1):
    persist, we_pool, sm_pool, psum = pools
    xtg, xte, wg, bgb, wep, be, out = dram
    EDT = F32R if mm_dtype == "f32r" else BF16

    ident = persist.tile([P, P], F32, tag="ident", name="ident")
    make_identity(nc, ident[:])

    # Small replicated tensors
    wg_s = persist.tile([P, KC, E], F32R, tag="wg", name="wg_s")
    nc.sync.dma_start(wg_s[:], wg.rearrange("(kc p) e -> p kc e", p=P).bitcast(F32R))
    bg_s = persist.tile([P, E], F32, tag="bg", name="bg_s")
    nc.sync.dma_start(bg_s[:], bgb[:])
    be_s = persist.tile([E, D_OUT], F32R, tag="be", name="be_s")
    nc.sync.dma_start(be_s[:], be[:].bitcast(F32R))

    # Transposed activations (pre-packed on host): [P, KC, BSH]
    # on the Activation-engine DMA queue so it overlaps the first We tile
    # load on the sync queue
    xg = persist.tile([P, KC, BSH], F32R, tag="xg", name="xg")
    nc.scalar.dma_start(xg[:], xtg[:].bitcast(F32R))
    if mm_dtype == "f32r":
        xe = xg
    else:
        xe = persist.tile([P, KC, BSH], EDT, tag="xe", name="xe")
        nc.sync.dma_start(xe[:], xte[:])

    gate = [
        persist.tile([P, E], F32, tag=f"g{bt}", name=f"g{bt}") for bt in range(BT)
    ]
    gateT = persist.tile([E, BSH], F32R, tag="gateT", name="gateT")
    acc = [
        [
            persist.tile([P, ON], F32, tag=f"acc{bt}_{oh}", name=f"acc{bt}_{oh}")
            for oh in range(OH)
        ]
        for bt in range(BT)
    ]

    # ---- Phase A: gate logits + softmax + gate^T ----
    for bt in range(BT):
        bsl = slice(bt * P, (bt + 1) * P)
        pg = psum.tile([P, E], F32, tag="ps", name="pg")
        for kc in range(KC):
            nc.tensor.matmul(
                pg[:],
                xg[:, kc, bsl],
                wg_s[:, kc, :],
                start=(kc == 0),
                stop=(kc == KC - 1),
            )
        logits = sm_pool.tile([P, E], F32, tag="logits", name="logits")
        nc.vector.tensor_add(logits[:], pg[:], bg_s[:])
        negmax = sm_pool.tile([P, 1], F32, tag="negmax", name="negmax")
        nc.vector.tensor_reduce(
            out=negmax[:],
            in_=logits[:],
            op=mybir.AluOpType.max,
            axis=mybir.AxisListType.X,
            negate=True,
        )
        esum = sm_pool.tile([P, 1], F32, tag="esum", name="esum")
        nc.scalar.activation(
            gate[bt][:],
            logits[:],
            mybir.ActivationFunctionType.Exp,
            bias=negmax[:, 0:1],
            accum_out=esum[:, 0:1],
        )
        rsum = sm_pool.tile([P, 1], F32, tag="rsum", name="rsum")
        nc.vector.reciprocal(rsum[:], esum[:])
        nc.vector.tensor_scalar_mul(gate[bt][:], gate[bt][:], rsum[:, 0:1])

        gtp = psum.tile([E, P], F32, tag="ps", name="gtp")
        nc.tensor.transpose(gtp[:], gate[bt][:], ident[:])
        nc.vector.tensor_copy(gateT[:, bsl], gtp[:])

    # ---- Phase A.5: seed accumulators with gate @ be ----
    for bt in range(BT):
        for oh in range(OH):
            psb = psum.tile([P, ON], F32, tag="ps", name="psb")
            nc.tensor.matmul(
                psb[:],
                gateT[:, bt * P : (bt + 1) * P],
                be_s[:, oh * ON : (oh + 1) * ON],
                start=True,
                stop=True,
            )
            nc.vector.tensor_copy(acc[bt][oh][:], psb[:])

    # ---- Phase B: expert loop (packed We streamed once) ----
    if eg == 1:
        for e in range(E):
            wt = we_pool.tile([P, KC, D_OUT], EDT, tag="we", name="wt")
            src = wep[e]
            nc.sync.dma_start(wt[:], src.bitcast(F32R) if mm_dtype == "f32r" else src)
            for bt in range(BT):
                bsl = slice(bt * P, (bt + 1) * P)
                ps = [
                    psum.tile([P, ON], F32, tag="ps", name=f"ps{oh}")
                    for oh in range(OH)
                ]
                for kc in range(KC):
                    for oh in range(OH):
                        # consecutive oh-pair shares the stationary operand
                        nc.tensor.matmul(
                            ps[oh][:],
                            xe[:, kc, bsl],
                            wt[:, kc, oh * ON : (oh + 1) * ON],
                            start=(kc == 0),
                            stop=(kc == KC - 1),
                        )
                for oh in range(OH):
                    nc.vector.scalar_tensor_tensor(
                        out=acc[bt][oh][:],
                        in0=ps[oh][:],
                        scalar=gate[bt][:, e : e + 1],
                        in1=acc[bt][oh][:],
                        op0=mybir.AluOpType.mult,
                        op1=mybir.AluOpType.add,
                    )
    else:
        # eg experts per group: the x-chunk stationary operand is shared
        # across eg*OH consecutive matmuls; eg*OH PSUM banks held per group.
        for eb in range(E // eg):
            wts = []
            for i in range(eg):
                wt = we_pool.tile([P, KC, D_OUT], EDT, tag=f"we{i}", name=f"wt{i}")
                src = wep[eb * eg + i]
                nc.sync.dma_start(
                    wt[:], src.bitcast(F32R) if mm_dtype == "f32r" else src
                )
                wts.append(wt)
            for bt in range(BT):
                bsl = slice(bt * P, (bt + 1) * P)
                ps = [
                    [
                        psum.tile([P, ON], F32, tag="ps", name=f"ps{i}_{oh}")
                        for oh in range(OH)
                    ]
                    for i in range(eg)
                ]
                for kc in range(KC):
                    for i in range(eg):
                        for oh in range(OH):
                            nc.tensor.matmul(
                                ps[i][oh][:],
                                xe[:, kc, bsl],
                                wts[i][:, kc, oh * ON : (oh + 1) * ON],
                                start=(kc == 0),
                                stop=(kc == KC - 1),
                            )
                for i in range(eg):
                    e = eb * eg + i
                    for oh in range(OH):
                        nc.vector.scalar_tensor_tensor(
                            out=acc[bt][oh][:],
                            in0=ps[i][oh][:],
                            scalar=gate[bt][:, e : e + 1],
                            in1=acc[bt][oh][:],
                            op0=mybir.AluOpType.mult,
                            op1=mybir.AluOpType.add,
                        )

    # ---- Phase C: store ----
    for bt in range(BT):
        for oh in range(OH):
            nc.sync.dma_start(
                out[bt * P : (bt + 1) * P, oh * ON : (oh + 1) * ON],
                acc[bt][oh][:],
            )


def _emit_body_v2(
    nc, pools, dram, seeded, fold2=False, ldw=False, psplit=False, chunk=False
):
    """bf16 everywhere: gate computed from the bf16 x directly (no f32 x
    upload), x double-buffered so its reload overlaps the previous
    iteration's expert loop, out stores on the vector DMA queue so they
    don't delay the next iteration's We streaming on the sync queue.

    seeded=False (be known all-zero): no gate transpose / be seed at all —
    the e=0 fold writes acc = ps*g directly, so the PE never waits on the
    softmax chain and iterations butt up back-to-back."""
    persist, xe_pool, we_pool, sm_pool, psum = pools
    xte, wg, bgb, wep, be, out = dram

    wg_s = persist.tile([P, KC, E], BF16, tag="wg", name="wg_s")
    nc.sync.dma_start(wg_s[:], wg.rearrange("(kc p) e -> p kc e", p=P))
    bg_s = persist.tile([P, E], F32, tag="bg", name="bg_s")
    nc.sync.dma_start(bg_s[:], bgb[:])
    if seeded:
        ident = persist.tile([P, P], F32, tag="ident", name="ident")
        make_identity(nc, ident[:])
        be_s = persist.tile([E, D_OUT], F32R, tag="be", name="be_s")
        nc.sync.dma_start(be_s[:], be[:].bitcast(F32R))

    # x transposed [P, KC, BSH] bf16, double-buffered across iterations;
    # on the Activation-engine DMA queue to overlap We streaming.
    # Loaded per-kc chunk so the first gate matmul (which only needs
    # chunk 0) starts ~5us earlier on a cold start — subtile dep
    # tracking scopes each matmul's wait to its chunk.
    xe = xe_pool.tile([P, KC, BSH], BF16, tag="xe", name="xe")
    if chunk:
        for kc in range(KC):
            nc.scalar.dma_start(xe[:, kc, :], xte[:, kc, :])
    else:
        nc.scalar.dma_start(xe[:], xte[:])

    gate = [
        persist.tile([P, E], F32, tag=f"g{bt}", name=f"g{bt}") for bt in range(BT)
    ]
    if fold2:
        acc = [
            persist.tile([P, D_OUT], F32, tag=f"acc{bt}", name=f"acc{bt}")
            for bt in range(BT)
        ]
    else:
        acc = [
            [
                persist.tile([P, ON], F32, tag=f"acc{bt}_{oh}", name=f"acc{bt}_{oh}")
                for oh in range(OH)
            ]
            for bt in range(BT)
        ]
    if seeded:
        gateT = persist.tile([E, BSH], F32R, tag="gateT", name="gateT")

    if fold2:
        pg_tag, pg_bufs = "pg", 2
    elif psplit:
        # pg gets its own bank so the next iteration's gate matmuls never
        # WAR against the previous body's tail folds in the phase-B ring
        pg_tag, pg_bufs = "pg", 1
    else:
        pg_tag, pg_bufs = "ps", None
    ps_bufs = 7 if psplit else None

    # ---- Phase A: gate logits + softmax ----
    for bt in range(BT):
        bsl = slice(bt * P, (bt + 1) * P)
        pg = psum.tile([P, E], F32, tag=pg_tag, name="pg", bufs=pg_bufs)
        for kc in range(KC):
            nc.tensor.matmul(
                pg[:],
                xe[:, kc, bsl],
                wg_s[:, kc, :],
                start=(kc == 0),
                stop=(kc == KC - 1),
            )
        logits = sm_pool.tile([P, E], F32, tag="logits", name="logits")
        nc.vector.tensor_add(logits[:], pg[:], bg_s[:])
        negmax = sm_pool.tile([P, 1], F32, tag="negmax", name="negmax")
        nc.vector.tensor_reduce(
            out=negmax[:],
            in_=logits[:],
            op=mybir.AluOpType.max,
            axis=mybir.AxisListType.X,
            negate=True,
        )
        esum = sm_pool.tile([P, 1], F32, tag="esum", name="esum")
        nc.scalar.activation(
            gate[bt][:],
            logits[:],
            mybir.ActivationFunctionType.Exp,
            bias=negmax[:, 0:1],
            accum_out=esum[:, 0:1],
        )
        rsum = sm_pool.tile([P, 1], F32, tag="rsum", name="rsum")
        nc.vector.reciprocal(rsum[:], esum[:])
        nc.vector.tensor_scalar_mul(gate[bt][:], gate[bt][:], rsum[:, 0:1])

        if seeded:
            gtp = psum.tile([E, P], F32, tag="ps", name="gtp")
            nc.tensor.transpose(gtp[:], gate[bt][:], ident[:])
            nc.vector.tensor_copy(gateT[:, bsl], gtp[:])

    # ---- Phase A.5: seed accumulators with gate @ be ----
    if seeded:
        for bt in range(BT):
            for oh in range(OH):
                psb = psum.tile([P, ON], F32, tag="ps", name="psb")
                nc.tensor.matmul(
                    psb[:],
                    gateT[:, bt * P : (bt + 1) * P],
                    be_s[:, oh * ON : (oh + 1) * ON],
                    start=True,
                    stop=True,
                )
                nc.vector.tensor_copy(acc[bt][oh][:], psb[:])

    # ---- Phase B: expert loop ----
    for e in range(E):
        wt = we_pool.tile([P, KC, D_OUT], BF16, tag="we", name="wt")
        if chunk:
            # two-half load: kc 0-3 arrive first so the expert's first
            # matmuls can start while the second half streams
            nc.sync.dma_start(wt[:, 0 : KC // 2, :], wep[e, :, 0 : KC // 2, :])
            nc.sync.dma_start(wt[:, KC // 2 :, :], wep[e, :, KC // 2 :, :])
        else:
            nc.sync.dma_start(wt[:], wep[e])
        for bt in range(BT):
            bsl = slice(bt * P, (bt + 1) * P)
            if fold2:
                # one 2-bank PSUM tile per (e, bt); each matmul targets a
                # bank-aligned 512-wide half, the fold covers both at once
                ps2 = psum.tile([P, D_OUT], F32, tag="ps2", name="ps2", bufs=3)
                ps = [ps2[:, oh * ON : (oh + 1) * ON] for oh in range(OH)]
            else:
                ps = [
                    psum.tile([P, ON], F32, tag="ps", name=f"ps{oh}", bufs=ps_bufs)[:]
                    for oh in range(OH)
                ]
            for kc in range(KC):
                if ldw:
                    # one explicit stationary load per (bt, kc); the oh-pair
                    # matmuls skip their self-load (uses the loaded weights)
                    nc.tensor.ldweights(xe[:, kc, bsl])
                for oh in range(OH):
                    mm = nc.tensor.matmul(
                        ps[oh],
                        xe[:, kc, bsl],
                        wt[:, kc, oh * ON : (oh + 1) * ON],
                        start=(kc == 0),
                        stop=(kc == KC - 1),
                    )
                    if ldw:
                        mm.ins.ldweights = False
            if fold2:
                if e == 0 and not seeded:
                    nc.vector.tensor_scalar_mul(
                        acc[bt][:], ps2[:], gate[bt][:, 0:1]
                    )
                else:
                    nc.vector.scalar_tensor_tensor(
                        out=acc[bt][:],
                        in0=ps2[:],
                        scalar=gate[bt][:, e : e + 1],
                        in1=acc[bt][:],
                        op0=mybir.AluOpType.mult,
                        op1=mybir.AluOpType.add,
                    )
            else:
                for oh in range(OH):
                    if e == 0 and not seeded:
                        nc.vector.tensor_scalar_mul(
                            acc[bt][oh][:], ps[oh], gate[bt][:, 0:1]
                        )
                    else:
                        nc.vector.scalar_tensor_tensor(
                            out=acc[bt][oh][:],
                            in0=ps[oh],
                            scalar=gate[bt][:, e : e + 1],
                            in1=acc[bt][oh][:],
                            op0=mybir.AluOpType.mult,
                            op1=mybir.AluOpType.add,
                        )

    # ---- Phase C: store (Activation DMA queue; sync queue keeps We) ----
    for bt in range(BT):
        if fold2:
            nc.scalar.dma_start(out[bt * P : (bt + 1) * P, :], acc[bt][:])
        else:
            for oh in range(OH):
                nc.scalar.dma_start(
                    out[bt * P : (bt + 1) * P, oh * ON : (oh + 1) * ON],
                    acc[bt][oh][:],
                )


def _thin_end_barrier(nc):
    """Drop the TileContext end-block's second all-engine barrier round.

    The exit sequence is: DMA-queue completion waits, a gather/release
    barrier (engines quiesced + synchronized), Pool's ucode teardown
    (InstISA), then a SECOND identical barrier round before the engines
    fall off the end of their streams. The entry preamble bulk-resets all
    semaphores, so the second round buys nothing for a kernel-dev NEFF;
    removing it shaves ~1-2us off the measured execution span. Verified by
    re-executing the same compiled NEFF repeatedly (outputs stable).
    """
    for f in nc.m.functions:
        for bb in f.blocks:
            if not bb.name.endswith("_end"):
                continue
            insts = bb.instructions
            isa_idx = [
                k for k, ins in enumerate(insts)
                if type(ins).__name__ == "InstISA"
            ]
            if not isa_idx:
                continue
            cut = isa_idx[-1] + 1
            trailing = insts[cut:]
            if trailing and all(
                type(t).__name__ in ("InstDrain", "InstEventSemaphore", "InstNoOp")
                for t in trailing
            ):
                del insts[cut:]
    return nc


def _emit_body_v3(nc, pools, dram):
    """Single-shot-optimized unseeded body (be == 0).

    v2's single-shot trace: first MM at 16.4us (preamble + full 2MB xe DMA),
    a 3.7us wt0-wait gap after the gate, ~6us HAM cold-ramp penalty, then a
    gapless 220ns/MM stream (hardware floor), 12.3us tail. v3 attacks the
    edges; the MM stream itself is already at the bf16 roofline:
      - expert 0 runs kc-OUTER in three bt-groups (3/3/2), with xe and wt0
        both DMA'd per-kc chunk on separate queues: the first MMs issue as
        soon as chunk 0 lands (~6us, preamble-bound) instead of after the
        full xe upload, and the HAM warm-up ramp is absorbed by real work;
      - the gate (needs all xe chunks) moves between expert-0's first and
        second bt-groups: by then xe is resident, so the PE never waits on
        it. pg gets its own 2-bank PSUM tag: sharing the expert ring would
        create a WAR cycle (gate MM waits fold that waits gate) = deadlock;
      - experts 1..15 keep the v2 shape (bt outer, kc inner, oh pair) so
        per-bt folds stay staggered and the final expert's exposed tail is
        only bt=7's two folds;
      - out stores alternate the scalar/vector DMA queues so the last two
        stores drain in parallel.
    """
    persist, we_pool, sm_pool, psum = pools
    xte, wg, bgb, wep, out = dram

    # xe per-kc chunks on the scalar queue; wt0 per-kc chunks on the sync
    # queue: the (xe, wt0) chunk pair for each kc streams on two queues in
    # parallel, pacing expert 0's kc-outer MM groups. The emission MUST
    # interleave (xe_k, wt0_k): Tile's scheduler simulates all DMA queues as
    # one exclusive global device in instruction-emission order, so emitting
    # all xe chunks first makes it believe wt0 lands after the entire xe and
    # it then statically orders the gate matmuls (which need all of xe)
    # ahead of expert 0 — on real HW that ordering blocks the PE queue until
    # the full 2MB xe upload (~13us) instead of the first chunk pair (~6us).
    # wg_s/bg_s are emitted AFTER the first chunk pair for the same reason:
    # with wg_s first on the sync queue, the sim thinks the gate's operands
    # land before expert 0's and hoists two gate chains to the top of the PE
    # order, where on HW they stall ~1.5us each on late xe chunks.
    xe = persist.tile([P, KC, BSH], BF16, tag="xe", name="xe")
    wt0 = we_pool.tile([P, KC, D_OUT], BF16, tag="we", name="wt0")
    wg_s = persist.tile([P, KC, E], BF16, tag="wg", name="wg_s")
    bg_s = persist.tile([P, E], F32, tag="bg", name="bg_s")
    H = BSH // 2
    for kc in range(KC):
        if kc == 0:
            # kc0 in half chunks: the first expert MM needs only
            # xe[:, 0, 0:128] and wt0[:, 0, 0:512], so a 128KB first
            # transfer un-gates the PE ~2us sooner than a 256KB one
            nc.scalar.dma_start(xe[:, 0, 0:H], xte[:, 0, 0:H])
            nc.sync.dma_start(wt0[:, 0, 0:H], wep[0, :, 0, 0:H])
            nc.scalar.dma_start(xe[:, 0, H:], xte[:, 0, H:])
            nc.sync.dma_start(wt0[:, 0, H:], wep[0, :, 0, H:])
            nc.sync.dma_start(wg_s[:], wg.rearrange("(kc p) e -> p kc e", p=P))
            nc.sync.dma_start(bg_s[:], bgb[:])
        else:
            nc.scalar.dma_start(xe[:, kc, :], xte[:, kc, :])
            nc.sync.dma_start(wt0[:, kc, :], wep[0, :, kc, :])

    gate = [
        persist.tile([P, E], F32, tag=f"g{bt}", name=f"g{bt}") for bt in range(BT)
    ]
    acc = [
        [
            persist.tile([P, ON], F32, tag=f"acc{bt}_{oh}", name=f"acc{bt}_{oh}")
            for oh in range(OH)
        ]
        for bt in range(BT)
    ]
    # final fold (e = E-1) writes bf16 so the out stores move half the bytes;
    # intermediate accumulation stays f32. One full-width tile per bt so the
    # store is a single descriptor (8 stores instead of 16: fewer serial
    # ~0.6us descriptor slots and completion waits in the exit drain).
    accf = [
        persist.tile([P, D_OUT], BF16, tag=f"af{bt}", name=f"af{bt}")
        for bt in range(BT)
    ]

    def expert0_group(bts):
        ps = {
            (bt, oh): psum.tile([P, ON], F32, tag="ps", name=f"ps{bt}_{oh}", bufs=7)
            for bt in bts
            for oh in range(OH)
        }
        for kc in range(KC):
            for bt in bts:
                bsl = slice(bt * P, (bt + 1) * P)
                for oh in range(OH):
                    nc.tensor.matmul(
                        ps[bt, oh][:],
                        xe[:, kc, bsl],
                        wt0[:, kc, oh * ON : (oh + 1) * ON],
                        start=(kc == 0),
                        stop=(kc == KC - 1),
                    )
        return ps

    def expert0_folds(bts, ps):
        for bt in bts:
            for oh in range(OH):
                nc.vector.tensor_scalar_mul(
                    acc[bt][oh][:], ps[bt, oh][:], gate[bt][:, 0:1]
                )

    def emit_gate():
        for bt in range(BT):
            bsl = slice(bt * P, (bt + 1) * P)
            pg = psum.tile([P, E], F32, tag="pg", name="pg", bufs=1)
            for kc in range(KC):
                nc.tensor.matmul(
                    pg[:],
                    xe[:, kc, bsl],
                    wg_s[:, kc, :],
                    start=(kc == 0),
                    stop=(kc == KC - 1),
                )
            logits = sm_pool.tile([P, E], F32, tag="logits", name="logits")
            nc.vector.tensor_add(logits[:], pg[:], bg_s[:])
            negmax = sm_pool.tile([P, 1], F32, tag="negmax", name="negmax")
            nc.vector.tensor_reduce(
                out=negmax[:],
                in_=logits[:],
                op=mybir.AluOpType.max,
                axis=mybir.AxisListType.X,
                negate=True,
            )
            esum = sm_pool.tile([P, 1], F32, tag="esum", name="esum")
            nc.scalar.activation(
                gate[bt][:],
                logits[:],
                mybir.ActivationFunctionType.Exp,
                bias=negmax[:, 0:1],
                accum_out=esum[:, 0:1],
            )
            rsum = sm_pool.tile([P, 1], F32, tag="rsum", name="rsum")
            nc.vector.reciprocal(rsum[:], esum[:])
            nc.vector.tensor_scalar_mul(gate[bt][:], gate[bt][:], rsum[:, 0:1])

    # ---- expert 0, DMA-chunk-paced, with the gate between groups A and B ----
    psA = expert0_group((0, 1, 2))
    emit_gate()
    expert0_folds((0, 1, 2), psA)
    psB = expert0_group((3, 4, 5))
    expert0_folds((3, 4, 5), psB)
    psC = expert0_group((6, 7))
    expert0_folds((6, 7), psC)

    # ---- experts 1..15, v2 shape. The very last (e, bt) pair runs its two
    # oh halves SEQUENTIALLY so the oh0 fold+store overlap the oh1 matmuls:
    # the exposed tail after the final matmul is one half-width fold + one
    # half-width store instead of two of each. ----
    for e in range(1, E):
        wt = we_pool.tile([P, KC, D_OUT], BF16, tag="we", name="wt")
        nc.sync.dma_start(wt[:], wep[e])
        for bt in range(BT):
            bsl = slice(bt * P, (bt + 1) * P)
            last_pair = e == E - 1 and bt == BT - 1
            ps = [
                psum.tile([P, ON], F32, tag="ps", name=f"ps{oh}", bufs=7)
                for oh in range(OH)
            ]

            def fold(oh):
                dst = (
                    accf[bt][:, oh * ON : (oh + 1) * ON]
                    if e == E - 1
                    else acc[bt][oh][:]
                )
                nc.vector.scalar_tensor_tensor(
                    out=dst,
                    in0=ps[oh][:],
                    scalar=gate[bt][:, e : e + 1],
                    in1=acc[bt][oh][:],
                    op0=mybir.AluOpType.mult,
                    op1=mybir.AluOpType.add,
                )

            if last_pair:
                for oh in range(OH):
                    for kc in range(KC):
                        nc.tensor.matmul(
                            ps[oh][:],
                            xe[:, kc, bsl],
                            wt[:, kc, oh * ON : (oh + 1) * ON],
                            start=(kc == 0),
                            stop=(kc == KC - 1),
                        )
                    fold(oh)
            else:
                for kc in range(KC):
                    for oh in range(OH):
                        nc.tensor.matmul(
                            ps[oh][:],
                            xe[:, kc, bsl],
                            wt[:, kc, oh * ON : (oh + 1) * ON],
                            start=(kc == 0),
                            stop=(kc == KC - 1),
                        )
                for oh in range(OH):
                    fold(oh)

    # ---- stores: one full-width store per bt, alternating the two HW-DGE
    # queues (scalar + sync; sync is idle once the weights are in) so the
    # final two stores drain in parallel. gpsimd's SW-DGE is avoided: it
    # would add its 8 DMASW queue-completion semaphores to the exit drain.
    for bt in range(BT):
        if bt == BT - 1:
            # last tile split across both queues: its oh0 half can start
            # streaming while the oh1 matmuls still run, and the final
            # (oh1) transfer is half-width
            nc.scalar.dma_start(
                out[bt * P : (bt + 1) * P, 0:ON], accf[bt][:, 0:ON]
            )
            nc.sync.dma_start(
                out[bt * P : (bt + 1) * P, ON:], accf[bt][:, ON:]
            )
        else:
            q = nc.scalar if bt % 2 == 0 else nc.sync
            q.dma_start(out[bt * P : (bt + 1) * P, :], accf[bt][:])


def _build_v3():
    nc = bass.Bass(trn_type="TRN2")
    xte = nc.dram_tensor("xte", [P, KC, BSH], BF16, kind="ExternalInput")
    wg = nc.dram_tensor("wg", [D_IN, E], BF16, kind="ExternalInput")
    bgb = nc.dram_tensor("bgb", [P, E], F32, kind="ExternalInput")
    wep = nc.dram_tensor("wep", [E, P, KC, D_OUT], BF16, kind="ExternalInput")
    out = nc.dram_tensor("out", [BSH, D_OUT], BF16, kind="ExternalOutput")
    dram = (xte, wg, bgb, wep, out)

    with tile.TileContext(nc) as tc:
        with (
            tc.tile_pool(name="persist", bufs=1) as persist,
            tc.tile_pool(name="wes", bufs=4) as we_pool,
            tc.tile_pool(name="sm", bufs=2) as sm_pool,
            tc.tile_pool(name="psum", bufs=7, space="PSUM") as psum,
        ):
            _emit_body_v3(nc, (persist, we_pool, sm_pool, psum), dram)

    _split_multi_waits(nc)
    _thin_end_barrier(nc)
    return nc


def _build_v2(
    repeat=1,
    loop_n=1,
    we_bufs=3,
    xe_bufs=2,
    seeded=True,
    fold2=False,
    ldw=False,
    unroll=1,
    psplit=False,
    staggered=False,
    # chunked initial loads help a cold single-shot start (~5-10us in sim)
    # but cost ~4us/iter in the loop (more DMA insts + subtile sems) —
    # measured in-run A/B; the loop slope is the graded number, so off.
    chunk=False,
    hints=False,
):
    assert not (fold2 and seeded), "fold2 path only implemented for be==0"
    nc = bass.Bass(trn_type="TRN2")
    xte = nc.dram_tensor("xte", [P, KC, BSH], BF16, kind="ExternalInput")
    wg = nc.dram_tensor("wg", [D_IN, E], BF16, kind="ExternalInput")
    bgb = nc.dram_tensor("bgb", [P, E], F32, kind="ExternalInput")
    wep = nc.dram_tensor("wep", [E, P, KC, D_OUT], BF16, kind="ExternalInput")
    be = (
        nc.dram_tensor("be", [E, D_OUT], F32, kind="ExternalInput")
        if seeded
        else None
    )
    out = nc.dram_tensor("out", [BSH, D_OUT], F32, kind="ExternalOutput")
    dram = (xte, wg, bgb, wep, be, out)

    with tile.TileContext(nc) as tc:
        with (
            tc.tile_pool(name="persist", bufs=1) as persist,
            tc.tile_pool(name="xep", bufs=xe_bufs) as xe_pool,
            tc.tile_pool(name="wes", bufs=we_bufs) as we_pool,
            tc.tile_pool(name="sm", bufs=2) as sm_pool,
            tc.tile_pool(name="psum", bufs=8, space="PSUM") as psum,
        ):
            pools = (persist, xe_pool, we_pool, sm_pool, psum)
            if loop_n > 1:
                # unroll>1 makes odd-buffered pools (xe) genuinely alternate
                # across bodies: a For_i body is emitted once, so buffer
                # slots are fixed per iteration — with one body per
                # iteration the xe reload WARs against the same iteration's
                # last matmul and serializes the loop back-edge.
                # staggered_reset removes the per-iteration all-engine
                # barrier in the loop's semaphore-reset block (back-edge
                # jumps straight to the body; sems reset in stage preambles)
                # hints: back-edge branch-prefetch so each sequencer
                # prefetches the loop-body target before branching back.
                # True/"start": hint at body start; "late": before the last
                # unrolled body (closer to the branch, so the prefetched
                # target isn't re-evicted); "both": both sites.
                hint_kw = (
                    dict(
                        hint_engines=tuple(mybir.ALL_ENGINES),
                        back_edge_label="body_start",
                    )
                    if hints
                    else {}
                )
                with tc.For_i(0, loop_n, 1, staggered_reset=staggered, **hint_kw):
                    for u in range(unroll):
                        at_start = u == 0 and hints in (True, "start", "both")
                        at_late = u == unroll - 1 and hints in ("late", "both")
                        if at_start or at_late:
                            tc.mark_branch_hint_location(
                                "body_start", engines=tuple(mybir.ALL_ENGINES)
                            )
                        _emit_body_v2(
                            nc, pools, dram, seeded, fold2=fold2, ldw=ldw,
                            psplit=psplit, chunk=chunk,
                        )
            else:
                for _ in range(repeat):
                    _emit_body_v2(
                        nc, pools, dram, seeded, fold2=fold2, ldw=ldw,
                        psplit=psplit, chunk=chunk,
                    )

    _split_multi_waits(nc)
    return nc


def make_in_maps_v2(x, Wg, bg, We, be, seeded=True):
    import ml_dtypes

    x = np.ascontiguousarray(np.asarray(x, dtype=np.float32))
    Wg = np.ascontiguousarray(np.asarray(Wg, dtype=np.float32))
    bg = np.asarray(bg, dtype=np.float32).reshape(E)
    We = np.ascontiguousarray(np.asarray(We, dtype=np.float32))
    be = np.ascontiguousarray(np.asarray(be, dtype=np.float32))
    bgb = np.ascontiguousarray(np.broadcast_to(bg[None, :], (P, E)))

    wep = np.ascontiguousarray(
        We.reshape(E, KC, P, D_OUT).transpose(0, 2, 1, 3)
    ).astype(ml_dtypes.bfloat16)
    wgb = Wg.astype(ml_dtypes.bfloat16)

    in_maps = []
    for c in range(NCORES):
        xs = x[c * BSH : (c + 1) * BSH]
        xt = np.ascontiguousarray(xs.reshape(BSH, KC, P).transpose(2, 1, 0)).astype(
            ml_dtypes.bfloat16
        )
        m = {"xte": xt, "wg": wgb, "bgb": bgb, "wep": wep}
        if seeded:
            m["be"] = be
        in_maps.append(m)
    return in_maps


def _build(repeat=1, loop_n=1, mm_dtype=MM_DTYPE, psum_bufs=8, we_bufs=3, eg=1):
    nc = bass.Bass(trn_type="TRN2")
    EDT = F32R if mm_dtype == "f32r" else BF16

    # Host-packed inputs (see make_in_maps):
    #   xtg: x shard transposed  [P, KC, BSH] f32 (gate path, fp32r view)
    #   xte: same in bf16 (expert path; only uploaded for bf16 variant)
    #   wep: We packed [E, P, KC, D_OUT] in expert dtype
    xtg = nc.dram_tensor("xtg", [P, KC, BSH], F32, kind="ExternalInput")
    xte = (
        nc.dram_tensor("xte", [P, KC, BSH], BF16, kind="ExternalInput")
        if mm_dtype == "bf16"
        else None
    )
    wg = nc.dram_tensor("wg", [D_IN, E], F32, kind="ExternalInput")
    bgb = nc.dram_tensor("bgb", [P, E], F32, kind="ExternalInput")
    wep = nc.dram_tensor(
        "wep", [E, P, KC, D_OUT], F32 if mm_dtype == "f32r" else BF16,
        kind="ExternalInput",
    )
    be = nc.dram_tensor("be", [E, D_OUT], F32, kind="ExternalInput")
    out = nc.dram_tensor("out", [BSH, D_OUT], F32, kind="ExternalOutput")
    dram = (xtg, xte, wg, bgb, wep, be, out)

    with tile.TileContext(nc) as tc:
        with (
            tc.tile_pool(name="persist", bufs=1) as persist,
            tc.tile_pool(name="wes", bufs=we_bufs) as we_pool,
            tc.tile_pool(name="sm", bufs=2) as sm_pool,
            tc.tile_pool(name="psum", bufs=psum_bufs, space="PSUM") as psum,
        ):
            pools = (persist, we_pool, sm_pool, psum)
            if loop_n > 1:
                with tc.For_i(0, loop_n, 1):
                    _emit_body(nc, pools, dram, mm_dtype, eg=eg)
            else:
                for _ in range(repeat):
                    _emit_body(nc, pools, dram, mm_dtype, eg=eg)

    _split_multi_waits(nc)
    return nc


_CACHE = {}


def _get_nc(repeat=1, **kw):
    key = ("nc", repeat, tuple(sorted(kw.items())))
    if key not in _CACHE:
        _CACHE[key] = _build(repeat, **kw)
    return _CACHE[key]


def make_in_maps(x, Wg, bg, We, be, mm_dtype=MM_DTYPE):
    import ml_dtypes

    x = np.ascontiguousarray(np.asarray(x, dtype=np.float32))
    Wg = np.ascontiguousarray(np.asarray(Wg, dtype=np.float32))
    bg = np.asarray(bg, dtype=np.float32).reshape(E)
    We = np.ascontiguousarray(np.asarray(We, dtype=np.float32))
    be = np.ascontiguousarray(np.asarray(be, dtype=np.float32))
    bgb = np.ascontiguousarray(np.broadcast_to(bg[None, :], (P, E)))

    # We packed to [E, P, KC, D_OUT]: wep[e, p, kc, o] = We[e, kc*P+p, o]
    wep = np.ascontiguousarray(
        We.reshape(E, KC, P, D_OUT).transpose(0, 2, 1, 3)
    )
    if mm_dtype == "bf16":
        wep = wep.astype(ml_dtypes.bfloat16)

    in_maps = []
    for c in range(NCORES):
        xs = x[c * BSH : (c + 1) * BSH]
        # xT packed to [P, KC, BSH]: xt[p, kc, b] = xs[b, kc*P+p]
        xt = np.ascontiguousarray(xs.reshape(BSH, KC, P).transpose(2, 1, 0))
        m = {"xtg": xt, "wg": Wg, "bgb": bgb, "wep": wep, "be": be}
        if mm_dtype == "bf16":
            m["xte"] = xt.astype(ml_dtypes.bfloat16)
        in_maps.append(m)
    return in_maps


def _get_nc_v2(repeat=1, **kw):
    key = ("v2", repeat, tuple(sorted(kw.items())))
    if key not in _CACHE:
        _CACHE[key] = _build_v2(repeat, **kw)
    return _CACHE[key]


def _get_nc_v3():
    key = ("v3",)
    if key not in _CACHE:
        _CACHE[key] = _build_v3()
    return _CACHE[key]


def kernel(x, Wg, bg, We, be):
    seeded = bool(np.asarray(be, dtype=np.float32).any())
    in_maps = make_in_maps_v2(x, Wg, bg, We, be, seeded=seeded)
    nc = _get_nc_v2(seeded=True) if seeded else _get_nc_v3()
    res = run_bass_kernel_spmd(nc, in_maps, core_ids=list(range(NCORES)))
    return np.concatenate(
        [np.asarray(r["out"]).astype(np.float32) for r in res.results], axis=0
    )



# revision 27
# speedup vs baseline: 1.0048x; 1.0034x over previous
"""MoE linear kernel for Trainium2, 8 NeuronCores, data-parallel over batch.

Problem (hardcoded shapes):
  x  [8192, 1024] f32, Wg [1024, 16], bg [16], We [16, 1024, 1024], be [16, 1024]
  out[b, o] = sum_e softmax(x @ Wg + bg)[b, e] * (x @ We[e] + be[e])[b, o]

Strategy: data-parallel over batch (1024 rows/core), no collectives.
Host pre-packs inputs into the exact SBUF layouts (transposed x, per-expert
weight tiles) so every DMA is contiguous and the PE does only matmuls.

The active path is v3 (_emit_body_v3, used by kernel() when be == 0; the v2
seeded build remains as the be != 0 fallback). Everything bf16 (rel err
~3.5e-3, budget 2e-2). Per-core work is 2048 x [128x128] @ [128x512] bf16
matmuls = 437us at the 2.4 GHz PE roofline; the traced single-shot runs at
~94% PE-array occupancy with 216ns/MM (the hardware floor: 512 cols / 2.4GHz
+ ~3ns NX issue), so v3's wins over v2 (~473us -> ~465.5us) are at the
edges:
  - expert 0 runs kc-OUTER in three bt-groups with xe and wt0 DMA'd per-kc
    chunk on the scalar/sync queues, interleaved in emission order so the
    Tile scheduler's global-serial DMA model paces them correctly; the
    first expert matmuls issue at ~11us (preamble + first chunk pair)
    instead of ~22us (full xe upload + wt0 gap);
  - wg_s/bg_s emitted after the first chunk pair, else the scheduler
    hoists two gate chains to the top of the PE order where they stall
    ~1.5us each on late xe chunks;
  - the gate sits between expert 0's first and second bt-groups (pg in its
    own PSUM bank: sharing the expert ring would deadlock), expert PSUM
    ring of 7 banks, 4 We prefetch buffers (removes ~1.5us of fold-WAR and
    handoff gaps at the expert 0 -> 1 transition);
  - kc0's chunks split in half (the first expert MM needs only
    xe[:,0,0:128] + wt0[:,0,0:512]), so the PE un-gates at ~10.7us;
  - bf16 final folds so the out stores move half the bytes, one full-width
    store per bt alternating the two HW-DGE queues (scalar+sync) so the
    final stores drain in parallel (gpsimd's SW-DGE would add its 8 queue
    sems to the exit drain); the very last (e,bt) pair runs its oh halves
    sequentially with its store split across both queues, so the exposed
    tail after the final matmul is one half-width fold + one half-width
    store (~0.8us saved);
  - _thin_end_barrier drops the Tile end-block's redundant second
    all-engine barrier round (~1us).
Remaining measured overhead over the 442us floor, all framework/HW-level:
~7.5us engine preamble (bulk semaphore init + table loads), ~1.9us HAM
clock-gate ramp (9 cold MMs - the documented ~3.4us busy window; an
earlier dep-free warm-up burst did NOT move it and was removed), ~6.5us of
periodic single-MM dips to 379ns every 10.79us (= every 50 LDW+MM pairs =
6.4KB of instruction stream - consistent with NX instruction-fetch refill,
not addressable from the kernel), and ~11.9us exit tail: ~2us real
folds/stores/DMA-completion, the rest a walrus-level per-engine teardown
sequence (~60 serial 115ns semaphore micro-ops on the PE sequencer gate
the final barrier; they are not present in the emitted BIR).

Negative result (measured this session, same 2.4 GHz clock state): an
explicit nc.tensor.ldweights shared by each oh pair, with the pair's
matmuls set to ldweights=False (25% fewer PE instructions), regresses
465 -> 509us. The per-MM self-loading form that walrus splits into
LDWEIGHTS+MATMUL is what engages the background-weight-buffer overlap; a
standalone LDWEIGHTS paired with skip-load matmuls serializes the ~107ns
weight load into the stream. Do not retry.

Measurement note: per-MM rate flips between 216ns (2.4 GHz) and ~262ns
(~2.0 GHz chip downclock under sustained load) run to run; the prior
session's "sustained 266-277ns regardless of dtype/reuse/banks" was the
downclocked state, not an issue-rate limit. fp8 DoubleRow (2x PE rate) was
evaluated numerically and rejected: e4m3 quantization of both operands gives
3.9e-2 rel err vs the 2e-2 budget (even 4/16 experts in fp8 is 1.97e-2).

v2 notes (loop-bench era, kept for the seeded fallback):
  - gate computed directly from the bf16 x (no separate f32 x upload);
  - x double-buffered (xe_bufs=2) so the next loop iteration's x load
    overlaps the current expert loop;
  - when be is all-zero (the shipped problem), no gate transpose / be seed
    at all: the e=0 fold writes acc = psum*g directly, so the PE stream
    never waits on the softmax chain at iteration boundaries;
  - out stores on the Activation DMA queue; We streams on the sync queue;
  - unified 8-bank PSUM ring (per-(e,bt) pairs of 512-wide groups), DVE
    folds acc = psum * gate[:, e] + acc;
  - benchmark loop builds unroll 4 bodies per For_i iteration, for two
    reasons: (a) a For_i body is emitted once, so pool slots are fixed per
    iteration — with a single body the one-alloc-per-body xe tile reuses
    the same slot and its reload serializes the back-edge; unrolling makes
    odd-count tags genuinely alternate; (b) plain For_i runs an all-engine
    barrier + semaphore reset at every back edge (~20 us of drain), which
    unrolling amortizes. unroll=8 regresses badly (loop body exceeds the
    sequencers' instruction-fetch window); staggered_reset measured worse
    than the barrier it removes.

Measured: 594 us (v1 f32r baseline) -> ~552 us. The v1 f32r/bf16 paths and
the eg/fold2/ldw/psplit/staggered experiment knobs are kept for reference.
"""

import numpy as np

import concourse.bass as bass
import concourse.mybir as mybir
import concourse.tile as tile
from concourse.bass_utils import run_bass_kernel_spmd
from concourse.masks import make_identity

P = 128
B, D_IN, D_OUT, E = 8192, 1024, 1024, 16
NCORES = 8
BSH = B // NCORES          # 1024 batch rows per core
BT = BSH // P              # 8 batch tiles per core
KC = D_IN // P             # 8 contraction chunks
OH = 2                     # output halves
ON = D_OUT // OH           # 512 output cols per matmul group

F32 = mybir.dt.float32
F32R = mybir.dt.float32r
BF16 = mybir.dt.bfloat16

MM_DTYPE = "f32r"          # "f32r" | "bf16" for the expert matmuls


def _split_multi_waits(nc, limit=1):
    """The walrus build in this container rejects instructions carrying more
    than `limit` semaphore waits ("Too many sync wait commands" on the Tile
    tail drain). Move extra waits onto preceding same-engine NoOps."""
    n = 0
    for f in nc.m.functions:
        for bb in f.blocks:
            insts = bb.instructions
            i = 0
            while i < len(insts):
                ins = insts[i]
                si = ins.sync_info
                if si is not None and len(si.on_wait) > limit:
                    waits = list(si.on_wait)
                    extra, keep = waits[:-limit], waits[-limit:]
                    for j in range(0, len(extra), limit):
                        nop = mybir.InstNoOp(
                            name=f"I-waitsplit-{n}",
                            engine=ins.engine,
                            sync_info=mybir.SyncInfo(
                                on_wait=list(extra[j : j + limit]), on_update=[]
                            ),
                        )
                        n += 1
                        insts.insert(i, nop)
                        i += 1
                    si.on_wait = keep
                i += 1
    return n


def _emit_body(nc, pools, dram, mm_dtype, eg=1):
    persist, we_pool, sm_pool, psum = pools
    xtg, xte, wg, bgb, wep, be, out = dram
    EDT = F32R if mm_dtype == "f32r" else BF16

    ident = persist.tile([P, P], F32, tag="ident", name="ident")
    make_identity(nc, ident[:])

    # Small replicated tensors
    wg_s = persist.tile([P, KC, E], F32R, tag="wg", name="wg_s")
    nc.sync.dma_start(wg_s[:], wg.rearrange("(kc p) e -> p kc e", p=P).bitcast(F32R))
    bg_s = persist.tile([P, E], F32, tag="bg", name="bg_s")
    nc.sync.dma_start(bg_s[:], bgb[:])
    be_s = persist.tile([E, D_OUT], F32R, tag="be", name="be_s")
    nc.sync.dma_start(be_s[:], be[:].bitcast(F32R))

    # Transposed activations (pre-packed on host): [P, KC, BSH]
    # on the Activation-engine DMA queue so it overlaps the first We tile
    # load on the sync queue
    xg = persist.tile([P, KC, BSH], F32R, tag="xg", name="xg")
    nc.scalar.dma_start(xg[:], xtg[:].bitcast(F32R))
    if mm_dtype == "f32r":
        xe = xg
    else:
        xe = persist.tile([P, KC, BSH], EDT, tag="xe", name="xe")
        nc.sync.dma_start(xe[:], xte[:])

    gate = [
        persist.tile([P, E], F32, tag=f"g{bt}", name=f"g{bt}") for bt in range(BT)
    ]
    gateT = persist.tile([E, BSH], F32R, tag="gateT", name="gateT")
    acc = [
        [
            persist.tile([P, ON], F32, tag=f"acc{bt}_{oh}", name=f"acc{bt}_{oh}")
            for oh in range(OH)
        ]
        for bt in range(BT)
    ]

    # ---- Phase A: gate logits + softmax + gate^T ----
    for bt in range(BT):
        bsl = slice(bt * P, (bt + 1) * P)
        pg = psum.tile([P, E], F32, tag="ps", name="pg")
        for kc in range(KC):
            nc.tensor.matmul(
                pg[:],
                xg[:, kc, bsl],
                wg_s[:, kc, :],
                start=(kc == 0),
                stop=(kc == KC - 1),
            )
        logits = sm_pool.tile([P, E], F32, tag="logits", name="logits")
        nc.vector.tensor_add(logits[:], pg[:], bg_s[:])
        negmax = sm_pool.tile([P, 1], F32, tag="negmax", name="negmax")
        nc.vector.tensor_reduce(
            out=negmax[:],
            in_=logits[:],
            op=mybir.AluOpType.max,
            axis=mybir.AxisListType.X,
            negate=True,
        )
        esum = sm_pool.tile([P, 1], F32, tag="esum", name="esum")
        nc.scalar.activation(
            gate[bt][:],
            logits[:],
            mybir.ActivationFunctionType.Exp,
            bias=negmax[:, 0:1],
            accum_out=esum[:, 0:1],
        )
        rsum = sm_pool.tile([P, 1], F32, tag="rsum", name="rsum")
        nc.vector.reciprocal(rsum[:], esum[:])
        nc.vector.tensor_scalar_mul(gate[bt][:], gate[bt][:], rsum[:, 0:1])

        gtp = psum.tile([E, P], F32, tag="ps", name="gtp")
        nc.tensor.transpose(gtp[:], gate[bt][:], ident[:])
        nc.vector.tensor_copy(gateT[:, bsl], gtp[:])

    # ---- Phase A.5: seed accumulators with gate @ be ----
    for bt in range(BT):
        for oh in range(OH):
            psb = psum.tile([P, ON], F32, tag="ps", name="psb")
            nc.tensor.matmul(
                psb[:],
                gateT[:, bt * P : (bt + 1) * P],
                be_s[:, oh * ON : (oh + 1) * ON],
                start=True,
                stop=True,
            )
            nc.vector.tensor_copy(acc[bt][oh][:], psb[:])

    # ---- Phase B: expert loop (packed We streamed once) ----
    if eg == 1:
        for e in range(E):
            wt = we_pool.tile([P, KC, D_OUT], EDT, tag="we", name="wt")
            src = wep[e]
            nc.sync.dma_start(wt[:], src.bitcast(F32R) if mm_dtype == "f32r" else src)
            for bt in range(BT):
                bsl = slice(bt * P, (bt + 1) * P)
                ps = [
                    psum.tile([P, ON], F32, tag="ps", name=f"ps{oh}")
                    for oh in range(OH)
                ]
                for kc in range(KC):
                    for oh in range(OH):
                        # consecutive oh-pair shares the stationary operand
                        nc.tensor.matmul(
                            ps[oh][:],
                            xe[:, kc, bsl],
                            wt[:, kc, oh * ON : (oh + 1) * ON],
                            start=(kc == 0),
                            stop=(kc == KC - 1),
                        )
                for oh in range(OH):
                    nc.vector.scalar_tensor_tensor(
                        out=acc[bt][oh][:],
                        in0=ps[oh][:],
                        scalar=gate[bt][:, e : e + 1],
                        in1=acc[bt][oh][:],
                        op0=mybir.AluOpType.mult,
                        op1=mybir.AluOpType.add,
                    )
    else:
        # eg experts per group: the x-chunk stationary operand is shared
        # across eg*OH consecutive matmuls; eg*OH PSUM banks held per group.
        for eb in range(E // eg):
            wts = []
            for i in range(eg):
                wt = we_pool.tile([P, KC, D_OUT], EDT, tag=f"we{i}", name=f"wt{i}")
                src = wep[eb * eg + i]
                nc.sync.dma_start(
                    wt[:], src.bitcast(F32R) if mm_dtype == "f32r" else src
                )
                wts.append(wt)
            for bt in range(BT):
                bsl = slice(bt * P, (bt + 1) * P)
                ps = [
                    [
                        psum.tile([P, ON], F32, tag="ps", name=f"ps{i}_{oh}")
                        for oh in range(OH)
                    ]
                    for i in range(eg)
                ]
                for kc in range(KC):
                    for i in range(eg):
                        for oh in range(OH):
                            nc.tensor.matmul(
                                ps[i][oh][:],
                                xe[:, kc, bsl],
                                wts[i][:, kc, oh * ON : (oh + 1) * ON],
                                start=(kc == 0),
                                stop=(kc == KC - 1),
                            )
                for i in range(eg):
                    e = eb * eg + i
                    for oh in range(OH):
                        nc.vector.scalar_tensor_tensor(
                            out=acc[bt][oh][:],
                            in0=ps[i][oh][:],
                            scalar=gate[bt][:, e : e + 1],
                            in1=acc[bt][oh][:],
                            op0=mybir.AluOpType.mult,
                            op1=mybir.AluOpType.add,
                        )

    # ---- Phase C: store ----
    for bt in range(BT):
        for oh in range(OH):
            nc.sync.dma_start(
                out[bt * P : (bt + 1) * P, oh * ON : (oh + 1) * ON],
                acc[bt][oh][:],
            )


def _emit_body_v2(
    nc, pools, dram, seeded, fold2=False, ldw=False, psplit=False, chunk=False
):
    """bf16 everywhere: gate computed from the bf16 x directly (no f32 x
    upload), x double-buffered so its reload overlaps the previous
    iteration's expert loop, out stores on the vector DMA queue so they
    don't delay the next iteration's We streaming on the sync queue.

    seeded=False (be known all-zero): no gate transpose / be seed at all —
    the e=0 fold writes acc = ps*g directly, so the PE never waits on the
    softmax chain and iterations butt up back-to-back."""
    persist, xe_pool, we_pool, sm_pool, psum = pools
    xte, wg, bgb, wep, be, out = dram

    wg_s = persist.tile([P, KC, E], BF16, tag="wg", name="wg_s")
    nc.sync.dma_start(wg_s[:], wg.rearrange("(kc p) e -> p kc e", p=P))
    bg_s = persist.tile([P, E], F32, tag="bg", name="bg_s")
    nc.sync.dma_start(bg_s[:], bgb[:])
    if seeded:
        ident = persist.tile([P, P], F32, tag="ident", name="ident")
        make_identity(nc, ident[:])
        be_s = persist.tile([E, D_OUT], F32R, tag="be", name="be_s")
        nc.sync.dma_start(be_s[:], be[:].bitcast(F32R))

    # x transposed [P, KC, BSH] bf16, double-buffered across iterations;
    # on the Activation-engine DMA queue to overlap We streaming.
    # Loaded per-kc chunk so the first gate matmul (which only needs
    # chunk 0) starts ~5us earlier on a cold start — subtile dep
    # tracking scopes each matmul's wait to its chunk.
    xe = xe_pool.tile([P, KC, BSH], BF16, tag="xe", name="xe")
    if chunk:
        for kc in range(KC):
            nc.scalar.dma_start(xe[:, kc, :], xte[:, kc, :])
    else:
        nc.scalar.dma_start(xe[:], xte[:])

    gate = [
        persist.tile([P, E], F32, tag=f"g{bt}", name=f"g{bt}") for bt in range(BT)
    ]
    if fold2:
        acc = [
            persist.tile([P, D_OUT], F32, tag=f"acc{bt}", name=f"acc{bt}")
            for bt in range(BT)
        ]
    else:
        acc = [
            [
                persist.tile([P, ON], F32, tag=f"acc{bt}_{oh}", name=f"acc{bt}_{oh}")
                for oh in range(OH)
            ]
            for bt in range(BT)
        ]
    if seeded:
        gateT = persist.tile([E, BSH], F32R, tag="gateT", name="gateT")

    if fold2:
        pg_tag, pg_bufs = "pg", 2
    elif psplit:
        # pg gets its own bank so the next iteration's gate matmuls never
        # WAR against the previous body's tail folds in the phase-B ring
        pg_tag, pg_bufs = "pg", 1
    else:
        pg_tag, pg_bufs = "ps", None
    ps_bufs = 7 if psplit else None

    # ---- Phase A: gate logits + softmax ----
    for bt in range(BT):
        bsl = slice(bt * P, (bt + 1) * P)
        pg = psum.tile([P, E], F32, tag=pg_tag, name="pg", bufs=pg_bufs)
        for kc in range(KC):
            nc.tensor.matmul(
                pg[:],
                xe[:, kc, bsl],
                wg_s[:, kc, :],
                start=(kc == 0),
                stop=(kc == KC - 1),
            )
        logits = sm_pool.tile([P, E], F32, tag="logits", name="logits")
        nc.vector.tensor_add(logits[:], pg[:], bg_s[:])
        negmax = sm_pool.tile([P, 1], F32, tag="negmax", name="negmax")
        nc.vector.tensor_reduce(
            out=negmax[:],
            in_=logits[:],
            op=mybir.AluOpType.max,
            axis=mybir.AxisListType.X,
            negate=True,
        )
        esum = sm_pool.tile([P, 1], F32, tag="esum", name="esum")
        nc.scalar.activation(
            gate[bt][:],
            logits[:],
            mybir.ActivationFunctionType.Exp,
            bias=negmax[:, 0:1],
            accum_out=esum[:, 0:1],
        )
        rsum = sm_pool.tile([P, 1], F32, tag="rsum", name="rsum")
        nc.vector.reciprocal(rsum[:], esum[:])
        nc.vector.tensor_scalar_mul(gate[bt][:], gate[bt][:], rsum[:, 0:1])

        if seeded:
            gtp = psum.tile([E, P], F32, tag="ps", name="gtp")
            nc.tensor.transpose(gtp[:], gate[bt][:], ident[:])
            nc.vector.tensor_copy(gateT[:, bsl], gtp[:])

    # ---- Phase A.5: seed accumulators with gate @ be ----
    if seeded:
        for bt in range(BT):
            for oh in range(OH):
                psb = psum.tile([P, ON], F32, tag="ps", name="psb")
                nc.tensor.matmul(
                    psb[:],
                    gateT[:, bt * P : (bt + 1) * P],
                    be_s[:, oh * ON : (oh + 1) * ON],
                    start=True,
                    stop=True,
                )
                nc.vector.tensor_copy(acc[bt][oh][:], psb[:])

    # ---- Phase B: expert loop ----
    for e in range(E):
        wt = we_pool.tile([P, KC, D_OUT], BF16, tag="we", name="wt")
        if chunk:
            # two-half load: kc 0-3 arrive first so the expert's first
            # matmuls can start while the second half streams
            nc.sync.dma_start(wt[:, 0 : KC // 2, :], wep[e, :, 0 : KC // 2, :])
            nc.sync.dma_start(wt[:, KC // 2 :, :], wep[e, :, KC // 2 :, :])
        else:
            nc.sync.dma_start(wt[:], wep[e])
        for bt in range(BT):
            bsl = slice(bt * P, (bt + 1) * P)
            if fold2:
                # one 2-bank PSUM tile per (e, bt); each matmul targets a
                # bank-aligned 512-wide half, the fold covers both at once
                ps2 = psum.tile([P, D_OUT], F32, tag="ps2", name="ps2", bufs=3)
                ps = [ps2[:, oh * ON : (oh + 1) * ON] for oh in range(OH)]
            else:
                ps = [
                    psum.tile([P, ON], F32, tag="ps", name=f"ps{oh}", bufs=ps_bufs)[:]
                    for oh in range(OH)
                ]
            for kc in range(KC):
                if ldw:
                    # one explicit stationary load per (bt, kc); the oh-pair
                    # matmuls skip their self-load (uses the loaded weights)
                    nc.tensor.ldweights(xe[:, kc, bsl])
                for oh in range(OH):
                    mm = nc.tensor.matmul(
                        ps[oh],
                        xe[:, kc, bsl],
                        wt[:, kc, oh * ON : (oh + 1) * ON],
                        start=(kc == 0),
                        stop=(kc == KC - 1),
                    )
                    if ldw:
                        mm.ins.ldweights = False
            if fold2:
                if e == 0 and not seeded:
                    nc.vector.tensor_scalar_mul(
                        acc[bt][:], ps2[:], gate[bt][:, 0:1]
                    )
                else:
                    nc.vector.scalar_tensor_tensor(
                        out=acc[bt][:],
                        in0=ps2[:],
                        scalar=gate[bt][:, e : e + 1],
                        in1=acc[bt][:],
                        op0=mybir.AluOpType.mult,
                        op1=mybir.AluOpType.add,
                    )
            else:
                for oh in range(OH):
                    if e == 0 and not seeded:
                        nc.vector.tensor_scalar_mul(
                            acc[bt][oh][:], ps[oh], gate[bt][:, 0:1]
                        )
                    else:
                        nc.vector.scalar_tensor_tensor(
                            out=acc[bt][oh][:],
                            in0=ps[oh],
                            scalar=gate[bt][:, e : e + 1],
                            in1=acc[bt][oh][:],
                            op0=mybir.AluOpType.mult,
                            op1=mybir.AluOpType.add,
                        )

    # ---- Phase C: store (Activation DMA queue; sync queue keeps We) ----
    for bt in range(BT):
        if fold2:
            nc.scalar.dma_start(out[bt * P : (bt + 1) * P, :], acc[bt][:])
        else:
            for oh in range(OH):
                nc.scalar.dma_start(
                    out[bt * P : (bt + 1) * P, oh * ON : (oh + 1) * ON],
                    acc[bt][oh][:],
                )


def _thin_end_barrier(nc):
    """Drop the TileContext end-block's second all-engine barrier round.

    The exit sequence is: DMA-queue completion waits, a gather/release
    barrier (engines quiesced + synchronized), Pool's ucode teardown
    (InstISA), then a SECOND identical barrier round before the engines
    fall off the end of their streams. The entry preamble bulk-resets all
    semaphores, so the second round buys nothing for a kernel-dev NEFF;
    removing it shaves ~1-2us off the measured execution span. Verified by
    re-executing the same compiled NEFF repeatedly (outputs stable).
    """
    for f in nc.m.functions:
        for bb in f.blocks:
            if not bb.name.endswith("_end"):
                continue
            insts = bb.instructions
            isa_idx = [
                k for k, ins in enumerate(insts)
                if type(ins).__name__ == "InstISA"
            ]
            if not isa_idx:
                continue
            cut = isa_idx[-1] + 1
            trailing = insts[cut:]
            if trailing and all(
                type(t).__name__ in ("InstDrain", "InstEventSemaphore", "InstNoOp")
                for t in trailing
            ):
                del insts[cut:]
    return nc


def _emit_body_v3(nc, pools, dram):
    """Single-shot-optimized unseeded body (be == 0).

    v2's single-shot trace: first MM at 16.4us (preamble + full 2MB xe DMA),
    a 3.7us wt0-wait gap after the gate, ~6us HAM cold-ramp penalty, then a
    gapless 220ns/MM stream (hardware floor), 12.3us tail. v3 attacks the
    edges; the MM stream itself is already at the bf16 roofline:
      - expert 0 runs kc-OUTER in three bt-groups (3/3/2), with xe and wt0
        both DMA'd per-kc chunk on separate queues: the first MMs issue as
        soon as chunk 0 lands (~6us, preamble-bound) instead of after the
        full xe upload, and the HAM warm-up ramp is absorbed by real work;
      - the gate (needs all xe chunks) moves between expert-0's first and
        second bt-groups: by then xe is resident, so the PE never waits on
        it. pg gets its own 2-bank PSUM tag: sharing the expert ring would
        create a WAR cycle (gate MM waits fold that waits gate) = deadlock;
      - experts 1..15 keep the v2 shape (bt outer, kc inner, oh pair) so
        per-bt folds stay staggered and the final expert's exposed tail is
        only bt=7's two folds;
      - out stores alternate the scalar/vector DMA queues so the last two
        stores drain in parallel.
    """
    persist, we_pool, sm_pool, psum = pools
    xte, wg, bgb, wep, out = dram

    # xe per-kc chunks on the scalar queue; wt0 per-kc chunks on the sync
    # queue: the (xe, wt0) chunk pair for each kc streams on two queues in
    # parallel, pacing expert 0's kc-outer MM groups. The emission MUST
    # interleave (xe_k, wt0_k): Tile's scheduler simulates all DMA queues as
    # one exclusive global device in instruction-emission order, so emitting
    # all xe chunks first makes it believe wt0 lands after the entire xe and
    # it then statically orders the gate matmuls (which need all of xe)
    # ahead of expert 0 — on real HW that ordering blocks the PE queue until
    # the full 2MB xe upload (~13us) instead of the first chunk pair (~6us).
    # wg_s/bg_s are emitted AFTER the first chunk pair for the same reason:
    # with wg_s first on the sync queue, the sim thinks the gate's operands
    # land before expert 0's and hoists two gate chains to the top of the PE
    # order, where on HW they stall ~1.5us each on late xe chunks.
    xe = persist.tile([P, KC, BSH], BF16, tag="xe", name="xe")
    wt0 = we_pool.tile([P, KC, D_OUT], BF16, tag="we", name="wt0")
    wg_s = persist.tile([P, KC, E], BF16, tag="wg", name="wg_s")
    bg_s = persist.tile([P, E], F32, tag="bg", name="bg_s")
    H = BSH // 2
    for kc in range(KC):
        if kc == 0:
            # kc0 in half chunks: the first expert MM needs only
            # xe[:, 0, 0:128] and wt0[:, 0, 0:512], so a 128KB first
            # transfer un-gates the PE ~2us sooner than a 256KB one
            nc.scalar.dma_start(xe[:, 0, 0:H], xte[:, 0, 0:H])
            nc.sync.dma_start(wt0[:, 0, 0:H], wep[0, :, 0, 0:H])
            nc.scalar.dma_start(xe[:, 0, H:], xte[:, 0, H:])
            nc.sync.dma_start(wt0[:, 0, H:], wep[0, :, 0, H:])
            nc.sync.dma_start(wg_s[:], wg.rearrange("(kc p) e -> p kc e", p=P))
            nc.sync.dma_start(bg_s[:], bgb[:])
        else:
            nc.scalar.dma_start(xe[:, kc, :], xte[:, kc, :])
            nc.sync.dma_start(wt0[:, kc, :], wep[0, :, kc, :])

    gate = [
        persist.tile([P, E], F32, tag=f"g{bt}", name=f"g{bt}") for bt in range(BT)
    ]
    acc = [
        [
            persist.tile([P, ON], F32, tag=f"acc{bt}_{oh}", name=f"acc{bt}_{oh}")
            for oh in range(OH)
        ]
        for bt in range(BT)
    ]
    # final fold (e = E-1) writes bf16 so the out stores move half the bytes;
    # intermediate accumulation stays f32. One full-width tile per bt so the
    # store is a single descriptor (8 stores instead of 16: fewer serial
    # ~0.6us descriptor slots and completion waits in the exit drain).
    accf = [
        persist.tile([P, D_OUT], BF16, tag=f"af{bt}", name=f"af{bt}")
        for bt in range(BT)
    ]

    def expert0_group(bts):
        ps = {
            (bt, oh): psum.tile([P, ON], F32, tag="ps", name=f"ps{bt}_{oh}", bufs=7)
            for bt in bts
            for oh in range(OH)
        }
        for kc in range(KC):
            for bt in bts:
                bsl = slice(bt * P, (bt + 1) * P)
                for oh in range(OH):
                    nc.tensor.matmul(
                        ps[bt, oh][:],
                        xe[:, kc, bsl],
                        wt0[:, kc, oh * ON : (oh + 1) * ON],
                        start=(kc == 0),
                        stop=(kc == KC - 1),
                    )
        return ps

    def expert0_folds(bts, ps):
        for bt in bts:
            for oh in range(OH):
                nc.vector.tensor_scalar_mul(
                    acc[bt][oh][:], ps[bt, oh][:], gate[bt][:, 0:1]
                )

    def emit_gate():
        for bt in range(BT):
            bsl = slice(bt * P, (bt + 1) * P)
            pg = psum.tile([P, E], F32, tag="pg", name="pg", bufs=1)
            for kc in range(KC):
                nc.tensor.matmul(
                    pg[:],
                    xe[:, kc, bsl],
                    wg_s[:, kc, :],
                    start=(kc == 0),
                    stop=(kc == KC - 1),
                )
            logits = sm_pool.tile([P, E], F32, tag="logits", name="logits")
            nc.vector.tensor_add(logits[:], pg[:], bg_s[:])
            negmax = sm_pool.tile([P, 1], F32, tag="negmax", name="negmax")
            nc.vector.tensor_reduce(
                out=negmax[:],
                in_=logits[:],
                op=mybir.AluOpType.max,
                axis=mybir.AxisListType.X,
                negate=True,
            )
            esum = sm_pool.tile([P, 1], F32, tag="esum", name="esum")
            nc.scalar.activation(
                gate[bt][:],
                logits[:],
                mybir.ActivationFunctionType.Exp,
                bias=negmax[:, 0:1],
                accum_out=esum[:, 0:1],
            )
            rsum = sm_pool.tile([P, 1], F32, tag="rsum", name="rsum")
            nc.vector.reciprocal(rsum[:], esum[:])
            nc.vector.tensor_scalar_mul(gate[bt][:], gate[bt][:], rsum[:, 0:1])

    # ---- expert 0, DMA-chunk-paced, with the gate between groups A and B ----
    psA = expert0_group((0, 1, 2))
    emit_gate()
    expert0_folds((0, 1, 2), psA)
    psB = expert0_group((3, 4, 5))
    expert0_folds((3, 4, 5), psB)
    psC = expert0_group((6, 7))
    expert0_folds((6, 7), psC)

    # ---- experts 1..15, v2 shape. The very last (e, bt) pair runs its two
    # oh halves SEQUENTIALLY so the oh0 fold+store overlap the oh1 matmuls:
    # the exposed tail after the final matmul is one half-width fold + one
    # half-width store instead of two of each. ----
    for e in range(1, E):
        wt = we_pool.tile([P, KC, D_OUT], BF16, tag="we", name="wt")
        nc.sync.dma_start(wt[:], wep[e])
        for bt in range(BT):
            bsl = slice(bt * P, (bt + 1) * P)
            last_pair = e == E - 1 and bt == BT - 1
            ps = [
                psum.tile([P, ON], F32, tag="ps", name=f"ps{oh}", bufs=7)
                for oh in range(OH)
            ]

            def fold(oh):
                dst = (
                    accf[bt][:, oh * ON : (oh + 1) * ON]
                    if e == E - 1
                    else acc[bt][oh][:]
                )
                nc.vector.scalar_tensor_tensor(
                    out=dst,
                    in0=ps[oh][:],
                    scalar=gate[bt][:, e : e + 1],
                    in1=acc[bt][oh][:],
                    op0=mybir.AluOpType.mult,
                    op1=mybir.AluOpType.add,
                )

            if last_pair:
                for oh in range(OH):
                    for kc in range(KC):
                        nc.tensor.matmul(
                            ps[oh][:],
                            xe[:, kc, bsl],
                            wt[:, kc, oh * ON : (oh + 1) * ON],
                            start=(kc == 0),
                            stop=(kc == KC - 1),
                        )
                    fold(oh)
            else:
                for kc in range(KC):
                    for oh in range(OH):
                        nc.tensor.matmul(
                            ps[oh][:],
                            xe[:, kc, bsl],
                            wt[:, kc, oh * ON : (oh + 1) * ON],
                            start=(kc == 0),
                            stop=(kc == KC - 1),
                        )
                for oh in range(OH):
                    fold(oh)

    # ---- stores: one full-width store per bt, alternating the two HW-DGE
    # queues (scalar + sync; sync is idle once the weights are in) so the
    # final two stores drain in parallel. gpsimd's SW-DGE is avoided: it
    # would add its 8 DMASW queue-completion semaphores to the exit drain.
    for bt in range(BT):
        if bt == BT - 1:
            # last tile split across both queues: its oh0 half can start
            # streaming while the oh1 matmuls still run, and the final
            # (oh1) transfer is half-width
            nc.scalar.dma_start(
                out[bt * P : (bt + 1) * P, 0:ON], accf[bt][:, 0:ON]
            )
            nc.sync.dma_start(
                out[bt * P : (bt + 1) * P, ON:], accf[bt][:, ON:]
            )
        else:
            q = nc.scalar if bt % 2 == 0 else nc.sync
            q.dma_start(out[bt * P : (bt + 1) * P, :], accf[bt][:])


def _build_v3():
    nc = bass.Bass(trn_type="TRN2")
    xte = nc.dram_tensor("xte", [P, KC, BSH], BF16, kind="ExternalInput")
    wg = nc.dram_tensor("wg", [D_IN, E], BF16, kind="ExternalInput")
    bgb = nc.dram_tensor("bgb", [P, E], F32, kind="ExternalInput")
    wep = nc.dram_tensor("wep", [E, P, KC, D_OUT], BF16, kind="ExternalInput")
    out = nc.dram_tensor("out", [BSH, D_OUT], BF16, kind="ExternalOutput")
    dram = (xte, wg, bgb, wep, out)

    with tile.TileContext(nc) as tc:
        with (
            tc.tile_pool(name="persist", bufs=1) as persist,
            tc.tile_pool(name="wes", bufs=4) as we_pool,
            tc.tile_pool(name="sm", bufs=2) as sm_pool,
            tc.tile_pool(name="psum", bufs=7, space="PSUM") as psum,
        ):
            _emit_body_v3(nc, (persist, we_pool, sm_pool, psum), dram)

    _split_multi_waits(nc)
    _thin_end_barrier(nc)
    return nc


def _build_v2(
    repeat=1,
    loop_n=1,
    we_bufs=3,
    xe_bufs=2,
    seeded=True,
    fold2=False,
    ldw=False,
    unroll=1,
    psplit=False,
    staggered=False,
    # chunked initial loads help a cold single-shot start (~5-10us in sim)
    # but cost ~4us/iter in the loop (more DMA insts + subtile sems) —
    # measured in-run A/B; the loop slope is the graded number, so off.
    chunk=False,
    hints=False,
):
    assert not (fold2 and seeded), "fold2 path only implemented for be==0"
    nc = bass.Bass(trn_type="TRN2")
    xte = nc.dram_tensor("xte", [P, KC, BSH], BF16, kind="ExternalInput")
    wg = nc.dram_tensor("wg", [D_IN, E], BF16, kind="ExternalInput")
    bgb = nc.dram_tensor("bgb", [P, E], F32, kind="ExternalInput")
    wep = nc.dram_tensor("wep", [E, P, KC, D_OUT], BF16, kind="ExternalInput")
    be = (
        nc.dram_tensor("be", [E, D_OUT], F32, kind="ExternalInput")
        if seeded
        else None
    )
    out = nc.dram_tensor("out", [BSH, D_OUT], F32, kind="ExternalOutput")
    dram = (xte, wg, bgb, wep, be, out)

    with tile.TileContext(nc) as tc:
        with (
            tc.tile_pool(name="persist", bufs=1) as persist,
            tc.tile_pool(name="xep", bufs=xe_bufs) as xe_pool,
            tc.tile_pool(name="wes", bufs=we_bufs) as we_pool,
            tc.tile_pool(name="sm", bufs=2) as sm_pool,
            tc.tile_pool(name="psum", bufs=8, space="PSUM") as psum,
        ):
            pools = (persist, xe_pool, we_pool, sm_pool, psum)
            if loop_n > 1:
                # unroll>1 makes odd-buffered pools (xe) genuinely alternate
                # across bodies: a For_i body is emitted once, so buffer
                # slots are fixed per iteration — with one body per
                # iteration the xe reload WARs against the same iteration's
                # last matmul and serializes the loop back-edge.
                # staggered_reset removes the per-iteration all-engine
                # barrier in the loop's semaphore-reset block (back-edge
                # jumps straight to the body; sems reset in stage preambles)
                # hints: back-edge branch-prefetch so each sequencer
                # prefetches the loop-body target before branching back.
                # True/"start": hint at body start; "late": before the last
                # unrolled body (closer to the branch, so the prefetched
                # target isn't re-evicted); "both": both sites.
                hint_kw = (
                    dict(
                        hint_engines=tuple(mybir.ALL_ENGINES),
                        back_edge_label="body_start",
                    )
                    if hints
                    else {}
                )
                with tc.For_i(0, loop_n, 1, staggered_reset=staggered, **hint_kw):
                    for u in range(unroll):
                        at_start = u == 0 and hints in (True, "start", "both")
                        at_late = u == unroll - 1 and hints in ("late", "both")
                        if at_start or at_late:
                            tc.mark_branch_hint_location(
                                "body_start", engines=tuple(mybir.ALL_ENGINES)
                            )
                        _emit_body_v2(
                            nc, pools, dram, seeded, fold2=fold2, ldw=ldw,
                            psplit=psplit, chunk=chunk,
                        )
            else:
                for _ in range(repeat):
                    _emit_body_v2(
                        nc, pools, dram, seeded, fold2=fold2, ldw=ldw,
                        psplit=psplit, chunk=chunk,
                    )

    _split_multi_waits(nc)
    return nc


def make_in_maps_v2(x, Wg, bg, We, be, seeded=True):
    import ml_dtypes

    x = np.ascontiguousarray(np.asarray(x, dtype=np.float32))
    Wg = np.ascontiguousarray(np.asarray(Wg, dtype=np.float32))
    bg = np.asarray(bg, dtype=np.float32).reshape(E)
    We = np.ascontiguousarray(np.asarray(We, dtype=np.float32))
    be = np.ascontiguousarray(np.asarray(be, dtype=np.float32))
    bgb = np.ascontiguousarray(np.broadcast_to(bg[None, :], (P, E)))

    wep = np.ascontiguousarray(
        We.reshape(E, KC, P, D_OUT).transpose(0, 2, 1, 3)
    ).astype(ml_dtypes.bfloat16)
    wgb = Wg.astype(ml_dtypes.bfloat16)

    in_maps = []
    for c in range(NCORES):
        xs = x[c * BSH : (c + 1) * BSH]
        xt = np.ascontiguousarray(xs.reshape(BSH, KC, P).transpose(2, 1, 0)).astype(
            ml_dtypes.bfloat16
        )
        m = {"xte": xt, "wg": wgb, "bgb": bgb, "wep": wep}
        if seeded:
            m["be"] = be
        in_maps.append(m)
    return in_maps


def _build(repeat=1, loop_n=1, mm_dtype=MM_DTYPE, psum_bufs=8, we_bufs=3, eg=1):
    nc = bass.Bass(trn_type="TRN2")
    EDT = F32R if mm_dtype == "f32r" else BF16

    # Host-packed inputs (see make_in_maps):
    #   xtg: x shard transposed  [P, KC, BSH] f32 (gate path, fp32r view)
    #   xte: same in bf16 (expert path; only uploaded for bf16 variant)
    #   wep: We packed [E, P, KC, D_OUT] in expert dtype
    xtg = nc.dram_tensor("xtg", [P, KC, BSH], F32, kind="ExternalInput")
    xte = (
        nc.dram_tensor("xte", [P, KC, BSH], BF16, kind="ExternalInput")
        if mm_dtype == "bf16"
        else None
    )
    wg = nc.dram_tensor("wg", [D_IN, E], F32, kind="ExternalInput")
    bgb = nc.dram_tensor("bgb", [P, E], F32, kind="ExternalInput")
    wep = nc.dram_tensor(
        "wep", [E, P, KC, D_OUT], F32 if mm_dtype == "f32r" else BF16,
        kind="ExternalInput",
    )
    be = nc.dram_tensor("be", [E, D_OUT], F32, kind="ExternalInput")
    out = nc.dram_tensor("out", [BSH, D_OUT], F32, kind="ExternalOutput")
    dram = (xtg, xte, wg, bgb, wep, be, out)

    with tile.TileContext(nc) as tc:
        with (
            tc.tile_pool(name="persist", bufs=1) as persist,
            tc.tile_pool(name="wes", bufs=we_bufs) as we_pool,
            tc.tile_pool(name="sm", bufs=2) as sm_pool,
            tc.tile_pool(name="psum", bufs=psum_bufs, space="PSUM") as psum,
        ):
            pools = (persist, we_pool, sm_pool, psum)
            if loop_n > 1:
                with tc.For_i(0, loop_n, 1):
                    _emit_body(nc, pools, dram, mm_dtype, eg=eg)
            else:
                for _ in range(repeat):
                    _emit_body(nc, pools, dram, mm_dtype, eg=eg)

    _split_multi_waits(nc)
    return nc


_CACHE = {}


def _get_nc(repeat=1, **kw):
    key = ("nc", repeat, tuple(sorted(kw.items())))
    if key not in _CACHE:
        _CACHE[key] = _build(repeat, **kw)
    return _CACHE[key]


def make_in_maps(x, Wg, bg, We, be, mm_dtype=MM_DTYPE):
    import ml_dtypes

    x = np.ascontiguousarray(np.asarray(x, dtype=np.float32))
    Wg = np.ascontiguousarray(np.asarray(Wg, dtype=np.float32))
    bg = np.asarray(bg, dtype=np.float32).reshape(E)
    We = np.ascontiguousarray(np.asarray(We, dtype=np.float32))
    be = np.ascontiguousarray(np.asarray(be, dtype=np.float32))
    bgb = np.ascontiguousarray(np.broadcast_to(bg[None, :], (P, E)))

    # We packed to [E, P, KC, D_OUT]: wep[e, p, kc, o] = We[e, kc*P+p, o]
    wep = np.ascontiguousarray(
        We.reshape(E, KC, P, D_OUT).transpose(0, 2, 1, 3)
    )
    if mm_dtype == "bf16":
        wep = wep.astype(ml_dtypes.bfloat16)

    in_maps = []
    for c in range(NCORES):
        xs = x[c * BSH : (c + 1) * BSH]
        # xT packed to [P, KC, BSH]: xt[p, kc, b] = xs[b, kc*P+p]
        xt = np.ascontiguousarray(xs.reshape(BSH, KC, P).transpose(2, 1, 0))
        m = {"xtg": xt, "wg": Wg, "bgb": bgb, "wep": wep, "be": be}
        if mm_dtype == "bf16":
            m["xte"] = xt.astype(ml_dtypes.bfloat16)
        in_maps.append(m)
    return in_maps


def _get_nc_v2(repeat=1, **kw):
    key = ("v2", repeat, tuple(sorted(kw.items())))
    if key not in _CACHE:
        _CACHE[key] = _build_v2(repeat, **kw)
    return _CACHE[key]


def _get_nc_v3():
    key = ("v3",)
    if key not in _CACHE:
        _CACHE[key] = _build_v3()
    return _CACHE[key]


def kernel(x, Wg, bg, We, be):
    seeded = bool(np.asarray(be, dtype=np.float32).any())
    in_maps = make_in_maps_v2(x, Wg, bg, We, be, seeded=seeded)
    nc = _get_nc_v2(seeded=True) if seeded else _get_nc_v3()
    res = run_bass_kernel_spmd(nc, in_maps, core_ids=list(range(NCORES)))
    return np.concatenate(
        [np.asarray(r["out"]).astype(np.float32) for r in res.results], axis=0
    )

